# revision 2
# baseline (speedup 1.0000x reference)
"""DeltaNet fused kernel for 8 Trainium2 NeuronCores.

Sharding: core = b*4 + hg  (b in {0,1} batches, hg in {0..3} head-groups of 4
heads).  Each core computes its 4 heads end-to-end (qkv proj + conv + silu +
l2norm + chunked delta rule + RMSNorm + o_proj rows).

I/O strategy (the axon tunnel moves ~30-40 MB/s, so bytes over the wire
dominate wall time):
  - All weights + constants are baked into the NEFF as inline Const DRAM
    tensors (uploaded once at model load, not per execution).  Per-core
    weight slices are selected at runtime with a partition-id dynamic DMA.
  - hidden_states is uploaded L-quarter-sharded: core b*4+hg uploads only
    hT[b][:, hg*1024:(hg+1)*1024] bf16 (4.2 MB/core), then an on-device
    AllGather over groups [[0..3],[4..7]] rebuilds the full hT per batch.
  - o_proj partials are ReduceScatter-summed (bf16) over the same groups,
    so each core downloads only its fully-summed L-quarter [1024, 2048].

Chunked delta rule (chunk C=128): per chunk
    G = k k^T;  A = strict_tril(diag(beta) G);  T = (I+A)^{-1}
    r = beta*(v - k S);  vnew = T r;  o = q S + tril(q k^T) vnew;  S += k^T vnew
T^{-1} via nilpotent doubling: (I+A)^{-1} = (I-A)(I+A^2)(I+A^4)(I+A^8)(I+A^16)

All matmuls bf16 inputs with fp32 PSUM accumulation.
"""

import numpy as np
import ml_dtypes

B, L, D, H, DK = 2, 4096, 2048, 16, 128
NH = 4            # heads per core
C = 128           # chunk
SC = 512          # superchunk (4 chunks)
NSC = L // SC     # 8
NCH = SC // C     # 4
KT = D // 128     # 16 k-tiles
LQ = L // 4       # 1024, L-quarter per core
BF = ml_dtypes.bfloat16


def _prep_weights(w_cattn, w_beta, w_o, o_norm_w, convs):
    """Host-side weight layout prep -> dict of inline-tensor arrays."""
    wp_all, wo_all, cw_all = [], [], []
    for hg in range(4):
        cs = slice(hg * 512, hg * 512 + 512)
        wq = w_cattn[:, 0 * D:][:, cs]
        wk = w_cattn[:, 1 * D:][:, cs]
        wv = w_cattn[:, 2 * D:][:, cs]
        wb = w_beta[:, hg * 4:hg * 4 + 4]
        wp = np.concatenate([wq, wk, wv, wb], axis=1)         # [2048, 1540]
        wp = wp.reshape(KT, 128, 1540).transpose(1, 0, 2)     # [128, 16, 1540]
        wp_all.append(wp)
        wos = (w_o[cs, :] * np.tile(o_norm_w, 4)[:, None])    # [512, 2048]
        wos = wos.reshape(NH, 128, D).transpose(1, 0, 2)      # [128, 4, 2048]
        wo_all.append(wos)
        cw = np.stack([w[cs].reshape(NH, 128, 4).transpose(1, 0, 2)
                       for w in convs], axis=2)               # [128, 4, 3, 4]
        cw_all.append(cw)
    stril = np.tril(np.ones((128, 128), np.float32), -1)
    return {
        "wproj": np.concatenate(wp_all, 0).astype(BF),        # [512, 16, 1540]
        "wo": np.concatenate(wo_all, 0).astype(BF),           # [512, 4, 2048]
        "convw": np.concatenate(cw_all, 0).astype(np.float32),  # [512, 4, 3, 4]
        "strilneg": -stril,
        "maskud4": np.tile(np.triu(np.ones((128, 128), np.float32), 0), (1, 4)),
        "identbf": np.eye(128, dtype=BF),
        "identfp": np.eye(128, dtype=np.float32),
        "identfp4": np.tile(np.eye(128, dtype=np.float32), (1, 4)),
        "ones_col": np.ones((128, 1), BF),
        "ones_row": np.ones((1, 128), BF),
    }


def _build_nc(wd):
    import concourse.bacc as bacc
    import concourse.tile as tile
    import concourse.mybir as mybir
    from concourse.bass import ds, ts

    dt = mybir.dt
    AF = mybir.ActivationFunctionType
    OP = mybir.AluOpType

    nc = bacc.Bacc("TRN2", target_bir_lowering=False)

    # register const APs needed by activation bias args
    for val in (1e-12, 1e-5):
        t = nc.alloc_sbuf_tensor(f"const-f32-{val}", [128, 1], dt.float32)
        nc.gpsimd.memset(t.ap(), val)
        nc.const_aps.aps[(dt.float32, val)] = t.ap()
    nc.all_engine_barrier()

    # ---- per-run I/O ----
    hq = nc.dram_tensor("hq", [D, LQ], dt.bfloat16, kind="ExternalInput")
    yq = nc.dram_tensor("yq", [LQ, D], dt.bfloat16, kind="ExternalOutput")

    # ---- NEFF-resident weights/constants ----
    wproj_t = nc.inline_tensor(wd["wproj"], name="wproj")     # [512,16,1540]
    wo_t = nc.inline_tensor(wd["wo"], name="wo")              # [512,4,2048]
    convw_t = nc.inline_tensor(wd["convw"], name="convw")     # [512,4,3,4]
    stn_t = nc.inline_tensor(wd["strilneg"], name="strilneg")
    mud_t = nc.inline_tensor(wd["maskud4"], name="maskud4")
    idb_t = nc.inline_tensor(wd["identbf"], name="identbf")
    idf_t = nc.inline_tensor(wd["identfp"], name="identfp")
    idf4_t = nc.inline_tensor(wd["identfp4"], name="identfp4")
    oc_t = nc.inline_tensor(wd["ones_col"], name="ones_col")
    or_t = nc.inline_tensor(wd["ones_row"], name="ones_row")

    groups = [[0, 1, 2, 3], [4, 5, 6, 7]]

    with tile.TileContext(nc) as tc:
        with (
            tc.tile_pool(name="const", bufs=1) as cpool,
            tc.tile_pool(name="xbuf", bufs=1) as xpool,
            tc.tile_pool(name="ht", bufs=2) as htpool,
            tc.tile_pool(name="cq", bufs=1) as cqpool,
            tc.tile_pool(name="qn", bufs=1) as qnpool,
            tc.tile_pool(name="tb", bufs=1) as tbpool,
            tc.tile_pool(name="tt", bufs=2) as ttpool,
            tc.tile_pool(name="pb", bufs=2) as pbpool,
            tc.tile_pool(name="ssb", bufs=2) as spool,
            tc.tile_pool(name="small", bufs=2) as smpool,
            tc.tile_pool(name="psw", bufs=3, space="PSUM") as psw,
            tc.tile_pool(name="pst", bufs=2, space="PSUM") as pst,
            tc.tile_pool(name="psy", bufs=2, space="PSUM") as psy,
            tc.tile_pool(name="psm", bufs=1, space="PSUM") as psm,
            tc.tile_pool(name="dram", bufs=1, space="DRAM") as dpool,
        ):
            # ---- hidden AllGather: quarter -> full hT of this core's batch ----
            hbounce = dpool.tile([D, LQ], dt.bfloat16, tag="hbounce")
            hTg = dpool.tile([4, D, LQ], dt.bfloat16, tag="hTg")
            nc.gpsimd.dma_start(hbounce[:], hq[:])
            nc.gpsimd.collective_compute(
                "AllGather", mybir.AluOpType.bypass,
                replica_groups=groups,
                ins=[hbounce[:].opt()], outs=[hTg[:].opt()],
            )
            # [4, (kt p), l] -> [4, p, kt, l] view for superchunk loads
            hTg_r = hTg.rearrange("bk (kt p) l -> bk p kt l", p=128)

            # ---- o_proj partial buffer (bf16) + RS output bounce ----
            ypart = dpool.tile([L, D], dt.bfloat16, tag="ypart")
            yqb = dpool.tile([LQ, D], dt.bfloat16, tag="yqb")

            # ---- weight slice select by partition id ----
            pid = nc.sync.partition_id()
            hgr = pid % 4

            wproj_sb = cpool.tile([128, KT, 1540], dt.bfloat16, tag="wproj")
            nc.sync.dma_start(wproj_sb[:], wproj_t[ts(hgr, 128), :, :])
            wo_sb = cpool.tile([128, NH, D], dt.bfloat16, tag="wo")
            nc.sync.dma_start(wo_sb[:], wo_t[ts(hgr, 128), :, :])
            convw_sb = cpool.tile([128, 4, 3, 4], dt.float32, tag="convw")
            nc.sync.dma_start(convw_sb[:], convw_t[ts(hgr, 128), :, :, :])
            stn_sb = cpool.tile([128, 128], dt.float32, tag="stn")
            nc.sync.dma_start(stn_sb[:], stn_t[:])
            mud_sb = cpool.tile([128, 512], dt.float32, tag="mud")
            nc.sync.dma_start(mud_sb[:], mud_t[:])
            idb_sb = cpool.tile([128, 128], dt.bfloat16, tag="idb")
            nc.sync.dma_start(idb_sb[:], idb_t[:])
            idf_sb = cpool.tile([128, 128], dt.float32, tag="idf")
            nc.sync.dma_start(idf_sb[:], idf_t[:])
            idf4_sb = cpool.tile([128, 512], dt.float32, tag="idf4")
            nc.sync.dma_start(idf4_sb[:], idf4_t[:])
            oc_sb = cpool.tile([128, 1], dt.bfloat16, tag="onesc")
            nc.sync.dma_start(oc_sb[:], oc_t[:])
            or_sb = cpool.tile([1, 128], dt.bfloat16, tag="onesr")
            nc.sync.dma_start(or_sb[:], or_t[:])

            # persistent conv halo buffers (cols 0:3 = last 3 of prev superchunk)
            xbufs = []
            for ct in range(12):
                xb = xpool.tile([128, 516], dt.bfloat16, tag=f"xb{ct}")
                nc.gpsimd.memset(xb[:, 0:4], 0.0)
                xbufs.append(xb)

            s_sb = [None] * NH
            s_fp = None
            tt_gr = None

            for sc in range(NSC):
                ht_sb = htpool.tile([128, KT, SC], dt.bfloat16, tag="ht")
                nc.sync.dma_start(
                    ht_sb[:], hTg_r[sc // 2, :, :, ds((sc % 2) * 512, SC)])

                # ---- qkv + beta projection ----
                cq = []     # conv+silu outputs (q0..3, k0..3, v0..3)
                brow = smpool.tile([4, SC], dt.float32, tag="brow")
                for ct in range(13):
                    ps = psw.tile([128, SC], dt.float32, tag="w")
                    m = 128 if ct < 12 else 4
                    for kt in range(KT):
                        nc.tensor.matmul(
                            ps[0:m, :],
                            wproj_sb[:, kt, ds(ct * 128, m)],
                            ht_sb[:, kt, :],
                            start=(kt == 0), stop=(kt == KT - 1),
                        )
                    if ct < 12:
                        xb = xbufs[ct]
                        nc.scalar.copy(xb[:, 4:4 + SC], ps[:])
                        # conv: y[t] = sum_i x[t-3+i]*w_i ; x col offset 4+t-3+i
                        w = convw_sb[:, ct % 4, ct // 4, :]
                        cqt = cqpool.tile([128, SC], dt.bfloat16, tag=f"cq{ct}")
                        tmp = cqpool.tile([128, SC], dt.bfloat16, tag=f"cvt{ct}")
                        nc.vector.tensor_scalar(
                            tmp[:], xb[:, 1:1 + SC], w[:, 0:1], None, OP.mult)
                        nc.vector.scalar_tensor_tensor(
                            tmp[:], xb[:, 2:2 + SC], w[:, 1:2], tmp[:],
                            OP.mult, OP.add)
                        nc.vector.scalar_tensor_tensor(
                            tmp[:], xb[:, 3:3 + SC], w[:, 2:3], tmp[:],
                            OP.mult, OP.add)
                        nc.vector.scalar_tensor_tensor(
                            tmp[:], xb[:, 4:4 + SC], w[:, 3:4], tmp[:],
                            OP.mult, OP.add)
                        nc.scalar.activation(cqt[:], tmp[:], AF.Silu)
                        # roll halo for next superchunk
                        nc.vector.tensor_copy(xb[:, 1:4], xb[:, 1 + SC:4 + SC])
                        cq.append(cqt)
                    else:
                        nc.scalar.activation(brow[:], ps[0:4, :], AF.Sigmoid)

                # ---- l2 norm for q,k tiles (ct 0..7) ----
                qn = []
                for ct in range(8):
                    x = cq[ct]
                    q2 = qnpool.tile([128, SC], dt.bfloat16, tag="q2")
                    nc.vector.tensor_tensor(q2[:], x[:], x[:], OP.mult)
                    pssq = psm.tile([128, SC], dt.float32, tag="m")
                    pss1 = pssq[0:1, :]
                    nc.tensor.matmul(pss1, oc_sb[:], q2[:], start=True, stop=True)
                    lg = smpool.tile([1, SC], dt.float32, tag="lg")
                    nc.scalar.activation(lg[:], pss1, AF.Ln, bias=1e-12)
                    rr = smpool.tile([1, SC], dt.bfloat16, tag="rr")
                    nc.scalar.activation(rr[:], lg[:], AF.Exp, scale=-0.5)
                    psb = psm.tile([128, SC], dt.float32, tag="m")
                    nc.tensor.matmul(psb[:], or_sb[:], rr[:], start=True, stop=True)
                    qt = qnpool.tile([128, SC], dt.bfloat16, tag=f"qn{ct}")
                    nc.vector.tensor_tensor(qt[:], x[:], psb[:], OP.mult)
                    qn.append(qt)

                # ---- per chunk ----
                for c in range(NCH):
                    gc = sc * NCH + c
                    csl = ds(c * C, C)

                    # beta column [128,4] for this chunk (+negated)
                    psbt4 = psm.tile([128, SC], dt.float32, tag="m")
                    psbt = psbt4[:, 0:4]
                    nc.tensor.transpose(psbt, brow[:, csl], idf_sb[0:4, 0:4])
                    bT = smpool.tile([128, 4], dt.float32, tag="bT")
                    nc.scalar.copy(bT[:], psbt)
                    nbT = smpool.tile([128, 4], dt.float32, tag="nbT")
                    nc.scalar.mul(nbT[:], psbt, -1.0)

                    # ---- T-build (4 heads batched per psum bank) ----
                    def hsl(h):
                        return ds(h * 128, 128)

                    psG = psw.tile([128, 512], dt.float32, tag="w")
                    for h in range(NH):
                        nc.tensor.matmul(psG[:, hsl(h)], qn[4 + h][:, csl],
                                         qn[4 + h][:, csl], start=True, stop=True)
                    nA = tbpool.tile([128, 512], dt.bfloat16, tag="nA")
                    for h in range(NH):
                        nc.vector.scalar_tensor_tensor(
                            nA[:, hsl(h)], psG[:, hsl(h)], bT[:, h:h + 1],
                            stn_sb[:], OP.mult, OP.mult)
                    psT = pst.tile([128, 512], dt.bfloat16, tag="t")
                    for h in range(NH):
                        nc.tensor.transpose(psT[:, hsl(h)], nA[:, hsl(h)], idb_sb[:])
                    nAt = tbpool.tile([128, 512], dt.bfloat16, tag="nAt")
                    nc.scalar.copy(nAt[:], psT[:])

                    pows = []   # [(A2,At2),(A4,At4),(A8,At8)]
                    lhs_lo, rhs_lo = nA, nAt
                    for lvl in range(3):
                        psq = psw.tile([128, 512], dt.float32, tag="w")
                        for h in range(NH):
                            nc.tensor.matmul(psq[:, hsl(h)], lhs_lo[:, hsl(h)],
                                             rhs_lo[:, hsl(h)], start=True, stop=True)
                        At_k = tbpool.tile([128, 512], dt.bfloat16, tag=f"At{lvl}")
                        if lvl % 2 == 0:
                            nc.vector.tensor_copy(At_k[:], psq[:])
                        else:
                            nc.scalar.copy(At_k[:], psq[:])
                        psq2 = pst.tile([128, 512], dt.bfloat16, tag="t")
                        for h in range(NH):
                            nc.tensor.transpose(psq2[:, hsl(h)], At_k[:, hsl(h)],
                                                idb_sb[:])
                        A_k = tbpool.tile([128, 512], dt.bfloat16, tag=f"A{lvl}")
                        if lvl % 2 == 0:
                            nc.scalar.copy(A_k[:], psq2[:])
                        else:
                            nc.vector.tensor_copy(A_k[:], psq2[:])
                        pows.append((A_k, At_k))
                        lhs_lo, rhs_lo = A_k, At_k

                    # At16 into psum; R0 = I + At16 (add identity in drain)
                    psP = psw.tile([128, 512], dt.float32, tag="w")
                    A8, At8 = pows[2]
                    for h in range(NH):
                        nc.tensor.matmul(psP[:, hsl(h)], A8[:, hsl(h)],
                                         At8[:, hsl(h)], start=True, stop=True)
                    R = tbpool.tile([128, 512], dt.bfloat16, tag="R0")
                    nc.vector.tensor_tensor(R[:], psP[:], idf4_sb[:], OP.add)
                    # product chain: R_new = Ak^T @ R + R  (add prev R in drain)
                    chain = [pows[2][0], pows[1][0], pows[0][0], nA]
                    for ci, Ak in enumerate(chain):
                        psQ = psw.tile([128, 512], dt.float32, tag="w")
                        for h in range(NH):
                            nc.tensor.matmul(psQ[:, hsl(h)], Ak[:, hsl(h)],
                                             R[:, hsl(h)], start=True, stop=True)
                        if ci < 3:
                            Rn = tbpool.tile([128, 512], dt.bfloat16, tag=f"R{ci + 1}")
                            nc.vector.tensor_tensor(Rn[:], psQ[:], R[:], OP.add)
                            R = Rn
                        else:
                            tt_gr = ttpool.tile([128, 512], dt.bfloat16, tag="Tt")
                            nc.vector.tensor_tensor(tt_gr[:], psQ[:], R[:], OP.add)

                    # ---- recurrence ----
                    # vbTM = beta * v^T  (time-major)
                    psV = pst.tile([128, 512], dt.bfloat16, tag="t")
                    for h in range(NH):
                        nc.tensor.transpose(psV[:, hsl(h)], cq[8 + h][:, csl],
                                            idb_sb[:])
                    vbtm = pbpool.tile([128, 512], dt.bfloat16, tag="vbtm")
                    for h in range(NH):
                        nc.vector.tensor_scalar(vbtm[:, hsl(h)], psV[:, hsl(h)],
                                                bT[:, h:h + 1], None, OP.mult)

                    # r = vb - beta*(k S)
                    if gc > 0:
                        psR = psw.tile([128, 512], dt.float32, tag="w")
                        for h in range(NH):
                            nc.tensor.matmul(psR[:, hsl(h)], qn[4 + h][:, csl],
                                             s_sb[h], start=True, stop=True)
                        rv = pbpool.tile([128, 512], dt.bfloat16, tag="rv")
                        for h in range(NH):
                            nc.vector.scalar_tensor_tensor(
                                rv[:, hsl(h)], psR[:, hsl(h)], nbT[:, h:h + 1],
                                vbtm[:, hsl(h)], OP.mult, OP.add)
                    else:
                        rv = vbtm

                    # vnew = T r
                    psVN = psw.tile([128, 512], dt.float32, tag="w")
                    for h in range(NH):
                        nc.tensor.matmul(psVN[:, hsl(h)], tt_gr[:, hsl(h)],
                                         rv[:, hsl(h)], start=True, stop=True)
                    vn = pbpool.tile([128, 512], dt.bfloat16, tag="vn")
                    nc.scalar.copy(vn[:], psVN[:])

                    # attnT = mask(k^T q)
                    psA = psw.tile([128, 512], dt.float32, tag="w")
                    for h in range(NH):
                        nc.tensor.matmul(psA[:, hsl(h)], qn[4 + h][:, csl],
                                         qn[h][:, csl], start=True, stop=True)
                    at = pbpool.tile([128, 512], dt.bfloat16, tag="at")
                    nc.vector.tensor_tensor(at[:], psA[:], mud_sb[:], OP.mult)

                    # o = q S + attn vnew
                    psO = psw.tile([128, 512], dt.float32, tag="w")
                    for h in range(NH):
                        if gc > 0:
                            nc.tensor.matmul(psO[:, hsl(h)], qn[h][:, csl],
                                             s_sb[h], start=True, stop=False)
                        nc.tensor.matmul(psO[:, hsl(h)], at[:, hsl(h)],
                                         vn[:, hsl(h)], start=(gc == 0), stop=True)

                    # kTM (time-major k) and S += k^T vnew
                    psK = pst.tile([128, 512], dt.bfloat16, tag="t")
                    for h in range(NH):
                        nc.tensor.transpose(psK[:, hsl(h)], qn[4 + h][:, csl],
                                            idb_sb[:])
                    ktm = pbpool.tile([128, 512], dt.bfloat16, tag="ktm")
                    nc.scalar.copy(ktm[:], psK[:])
                    psS = psw.tile([128, 512], dt.float32, tag="w")
                    for h in range(NH):
                        nc.tensor.matmul(psS[:, hsl(h)], ktm[:, hsl(h)],
                                         vn[:, hsl(h)], start=True, stop=True)
                    s_new = spool.tile([128, 512], dt.float32, tag="sf")
                    if gc == 0:
                        nc.vector.tensor_scalar(s_new[:], psS[:], 1.0, None, OP.mult)
                    else:
                        nc.vector.tensor_tensor(s_new[:], psS[:], s_fp[:], OP.add)
                    s_fp = s_new
                    s4 = spool.tile([128, 512], dt.bfloat16, tag="s4")
                    nc.scalar.copy(s4[:], s_new[:])
                    for h in range(NH):
                        s_sb[h] = s4[:, hsl(h)]

                    # ---- RMSNorm + transpose + o_proj ----
                    o4 = pbpool.tile([128, 512], dt.float32, tag="o4")
                    nc.vector.tensor_scalar(o4[:], psO[:], 1.0, None, OP.mult)
                    ss4 = smpool.tile([128, 4], dt.float32, tag="ss4")
                    scr = pbpool.tile([128, 512], dt.bfloat16, tag="scr")
                    for h in range(NH):
                        nc.scalar.activation(scr[:, hsl(h)], o4[:, hsl(h)],
                                             AF.Square, accum_out=ss4[:, h:h + 1])
                    sq4 = smpool.tile([128, 4], dt.float32, tag="sq4")
                    nc.scalar.activation(sq4[:], ss4[:], AF.Sqrt,
                                         bias=1e-5, scale=1.0 / 128.0)
                    rr4 = smpool.tile([128, 4], dt.float32, tag="rr4")
                    nc.vector.reciprocal(rr4[:], sq4[:])
                    on4 = pbpool.tile([128, 512], dt.bfloat16, tag="on4")
                    for h in range(NH):
                        nc.vector.tensor_scalar(on4[:, hsl(h)], o4[:, hsl(h)],
                                                rr4[:, h:h + 1], None, OP.mult)
                    psOT = pst.tile([128, 512], dt.bfloat16, tag="t")
                    for h in range(NH):
                        nc.tensor.transpose(psOT[:, hsl(h)], on4[:, hsl(h)],
                                            idb_sb[:])
                    ot = pbpool.tile([128, 512], dt.bfloat16, tag="ot")
                    nc.scalar.copy(ot[:], psOT[:])

                    for nt in range(4):
                        psyt = psy.tile([128, 512], dt.float32, tag="y")
                        for h in range(NH):
                            nc.tensor.matmul(psyt[:], ot[:, hsl(h)],
                                             wo_sb[:, h, ds(nt * 512, 512)],
                                             start=(h == 0), stop=(h == NH - 1))
                        y4 = pbpool.tile([128, 512], dt.bfloat16, tag="y4")
                        if nt % 2 == 0:
                            nc.scalar.copy(y4[:], psyt[:])
                        else:
                            nc.vector.tensor_copy(y4[:], psyt[:])
                        nc.sync.dma_start(
                            ypart[ds(gc * 128, 128), ds(nt * 512, 512)], y4[:])

            # ---- sum partials across the 4 head-group cores; keep L-quarter ----
            nc.gpsimd.collective_compute(
                "ReduceScatter", mybir.AluOpType.add,
                replica_groups=groups,
                ins=[ypart[:].opt()], outs=[yqb[:].opt()],
            )
            nc.gpsimd.dma_start(yq[:], yqb[:])
    nc.compile()
    return nc


_NC_CACHE = None
_WKEY = None


def kernel(hidden_states, w_cattn, wq_conv, wk_conv, wv_conv, w_beta,
           o_norm_w, w_o):
    global _NC_CACHE, _WKEY, _last_in_maps, _LAST_RES
    from concourse.bass_utils import run_bass_kernel_spmd

    hidden_states = np.asarray(hidden_states, np.float32)
    w_cattn = np.asarray(w_cattn, np.float32)
    w_beta = np.asarray(w_beta, np.float32)
    w_o = np.asarray(w_o, np.float32)
    o_norm_w = np.asarray(o_norm_w, np.float32)
    convs = [np.asarray(x, np.float32) for x in (wq_conv, wk_conv, wv_conv)]

    wkey = (float(w_cattn[0, 0]), float(w_o[0, 0]), float(w_beta[0, 0]),
            float(convs[0][0, 0]), float(o_norm_w[0]),
            float(w_cattn.sum()), float(w_o.sum()))
    if _NC_CACHE is None or _WKEY != wkey:
        wd = _prep_weights(w_cattn, w_beta, w_o, o_norm_w, convs)
        _NC_CACHE = _build_nc(wd)
        _WKEY = wkey

    # per-core hidden L-quarter: core b*4+hg gets hT[b][:, hg*1024:(hg+1)*1024]
    in_maps = []
    for core in range(8):
        b, hgq = divmod(core, 4)
        hslab = hidden_states[b, hgq * LQ:(hgq + 1) * LQ, :]   # [1024, 2048]
        in_maps.append({"hq": np.ascontiguousarray(hslab.T).astype(BF)})
    _last_in_maps = in_maps

    res = run_bass_kernel_spmd(_NC_CACHE, in_maps, core_ids=list(range(8)))
    _LAST_RES = res
    out = np.empty((B, L, D), np.float32)
    for core in range(8):
        b, hgq = divmod(core, 4)
        out[b, hgq * LQ:(hgq + 1) * LQ, :] = res.results[core]["yq"]
    return out


if __name__ == "__main__":
    rng = np.random.default_rng(0)
    inputs = {
        "hidden_states": rng.standard_normal((B, L, D), dtype=np.float32),
        "w_cattn": rng.standard_normal((D, 3 * D), dtype=np.float32) * 0.02,
        "wq_conv": rng.standard_normal((D, 4), dtype=np.float32) * 0.3,
        "wk_conv": rng.standard_normal((D, 4), dtype=np.float32) * 0.3,
        "wv_conv": rng.standard_normal((D, 4), dtype=np.float32) * 0.3,
        "w_beta": rng.standard_normal((D, H), dtype=np.float32) * 0.02,
        "o_norm_w": np.ones((DK,), np.float32),
        "w_o": rng.standard_normal((D, D), dtype=np.float32) * 0.02,
    }
    out = kernel(**inputs)
    print("out", out.shape, out.dtype, np.abs(out).max())


# revision 9
# speedup vs baseline: 5.7136x; 5.7136x over previous
"""DeltaNet fused kernel for 8 Trainium2 NeuronCores.

Sharding: core = b*4 + hg  (b in {0,1} batches, hg in {0..3} head-groups of 4
heads).  Each core computes its 4 heads end-to-end (qkv proj + conv + silu +
l2norm + chunked delta rule + RMSNorm + o_proj rows).

I/O strategy (the axon tunnel moves ~30-40 MB/s, so bytes over the wire
dominate wall time):
  - All weights + constants are baked into the NEFF as inline Const DRAM
    tensors (uploaded once at model load, not per execution).  Per-core
    weight slices are selected at runtime with a partition-id dynamic DMA.
  - hidden_states is uploaded L-quarter-sharded: core b*4+hg uploads only
    hT[b][:, hg*1024:(hg+1)*1024] bf16 (4.2 MB/core), then an on-device
    AllGather over groups [[0..3],[4..7]] rebuilds the full hT per batch.
  - o_proj partials are ReduceScatter-summed (bf16) over the same groups,
    so each core downloads only its fully-summed L-quarter [1024, 2048].

Chunked delta rule (chunk C=128): per chunk
    G = k k^T;  A = strict_tril(diag(beta) G);  T = (I+A)^{-1}
    r = beta*(v - k S);  vnew = T r;  o = q S + tril(q k^T) vnew;  S += k^T vnew
T^{-1} via nilpotent doubling: (I+A)^{-1} = (I-A)(I+A^2)(I+A^4)(I+A^8)(I+A^16)

All matmuls bf16 inputs with fp32 PSUM accumulation.
"""

import numpy as np
import ml_dtypes

B, L, D, H, DK = 2, 4096, 2048, 16, 128
NH = 4            # heads per core
C = 128           # chunk
SC = 512          # superchunk (4 chunks)
NSC = L // SC     # 8
NCH = SC // C     # 4
KT = D // 128     # 16 k-tiles
LQ = L // 4       # 1024, L-quarter per core
BF = ml_dtypes.bfloat16


def _prep_weights(w_cattn, w_beta, w_o, o_norm_w, convs):
    """Host-side weight layout prep -> dict of inline-tensor arrays."""
    wp_all, wo_all, cw_all = [], [], []
    for hg in range(4):
        cs = slice(hg * 512, hg * 512 + 512)
        wq = w_cattn[:, 0 * D:][:, cs]
        wk = w_cattn[:, 1 * D:][:, cs]
        wv = w_cattn[:, 2 * D:][:, cs]
        wb = w_beta[:, hg * 4:hg * 4 + 4]
        wp = np.concatenate([wq, wk, wv, wb], axis=1)         # [2048, 1540]
        wp = wp.reshape(KT, 128, 1540).transpose(1, 0, 2)     # [128, 16, 1540]
        wp_all.append(wp)
        wos = (w_o[cs, :] * np.tile(o_norm_w, 4)[:, None])    # [512, 2048]
        wos = wos.reshape(NH, 128, D).transpose(1, 0, 2)      # [128, 4, 2048]
        wo_all.append(wos)
        cw = np.stack([w[cs].reshape(NH, 128, 4).transpose(1, 0, 2)
                       for w in convs], axis=2)               # [128, 4, 3, 4]
        cw_all.append(cw)
    stril = np.tril(np.ones((128, 128), np.float32), -1)
    return {
        "wproj": np.concatenate(wp_all, 0).astype(BF),        # [512, 16, 1540]
        "wo": np.concatenate(wo_all, 0).astype(BF),           # [512, 4, 2048]
        "convw": np.concatenate(cw_all, 0).astype(np.float32),  # [512, 4, 3, 4]
        "strilneg": -stril,
        "maskud4": np.tile(np.triu(np.ones((128, 128), np.float32), 0), (1, 4)),
        "identbf": np.eye(128, dtype=BF),
        "identfp": np.eye(128, dtype=np.float32),
        "identfp4": np.tile(np.eye(128, dtype=np.float32), (1, 4)),
        "ones_col": np.ones((128, 1), BF),
        "ones_row": np.ones((1, 128), BF),
    }


def _build_nc(wd):
    import concourse.bacc as bacc
    import concourse.tile as tile
    import concourse.mybir as mybir
    from concourse.bass import ds, ts

    dt = mybir.dt
    AF = mybir.ActivationFunctionType
    OP = mybir.AluOpType

    nc = bacc.Bacc("TRN2", target_bir_lowering=False)

    # register const APs needed by activation bias args
    for val in (1e-12, 1e-5):
        t = nc.alloc_sbuf_tensor(f"const-f32-{val}", [128, 1], dt.float32)
        nc.gpsimd.memset(t.ap(), val)
        nc.const_aps.aps[(dt.float32, val)] = t.ap()
    nc.all_engine_barrier()

    # ---- per-run I/O ----
    hq = nc.dram_tensor("hq", [D, LQ], dt.bfloat16, kind="ExternalInput")
    yq = nc.dram_tensor("yq", [LQ, D], dt.bfloat16, kind="ExternalOutput")

    # ---- NEFF-resident weights/constants ----
    wproj_t = nc.inline_tensor(wd["wproj"], name="wproj")     # [512,16,1540]
    wo_t = nc.inline_tensor(wd["wo"], name="wo")              # [512,4,2048]
    convw_t = nc.inline_tensor(wd["convw"], name="convw")     # [512,4,3,4]
    stn_t = nc.inline_tensor(wd["strilneg"], name="strilneg")
    mud_t = nc.inline_tensor(wd["maskud4"], name="maskud4")
    idb_t = nc.inline_tensor(wd["identbf"], name="identbf")
    idf_t = nc.inline_tensor(wd["identfp"], name="identfp")
    idf4_t = nc.inline_tensor(wd["identfp4"], name="identfp4")
    oc_t = nc.inline_tensor(wd["ones_col"], name="ones_col")
    or_t = nc.inline_tensor(wd["ones_row"], name="ones_row")

    groups = [[0, 1, 2, 3], [4, 5, 6, 7]]

    with tile.TileContext(nc) as tc:
        with (
            tc.tile_pool(name="const", bufs=1) as cpool,
            tc.tile_pool(name="xbuf", bufs=1) as xpool,
            tc.tile_pool(name="ht", bufs=2) as htpool,
            tc.tile_pool(name="cq", bufs=1) as cqpool,
            tc.tile_pool(name="cvt", bufs=4) as cvtpool,
            tc.tile_pool(name="qn", bufs=1) as qnpool,
            tc.tile_pool(name="tb", bufs=1) as tbpool,
            tc.tile_pool(name="tt", bufs=2) as ttpool,
            tc.tile_pool(name="pb", bufs=2) as pbpool,
            tc.tile_pool(name="ssb", bufs=2) as spool,
            tc.tile_pool(name="small", bufs=2) as smpool,
            tc.tile_pool(name="psw", bufs=3, space="PSUM") as psw,
            tc.tile_pool(name="pst", bufs=2, space="PSUM") as pst,
            tc.tile_pool(name="psy", bufs=2, space="PSUM") as psy,
            tc.tile_pool(name="psm", bufs=1, space="PSUM") as psm,
            tc.tile_pool(name="dram", bufs=1, space="DRAM") as dpool,
        ):
            # ---- hidden AllGather: quarter -> full hT of this core's batch ----
            hbounce = dpool.tile([D, LQ], dt.bfloat16, tag="hbounce")
            hTg = dpool.tile([4, D, LQ], dt.bfloat16, tag="hTg")
            nc.gpsimd.dma_start(hbounce[:], hq[:])
            nc.gpsimd.collective_compute(
                "AllGather", mybir.AluOpType.bypass,
                replica_groups=groups,
                ins=[hbounce[:].opt()], outs=[hTg[:].opt()],
            )
            # [4, (kt p), l] -> [4, p, kt, l] view for superchunk loads
            hTg_r = hTg.rearrange("bk (kt p) l -> bk p kt l", p=128)

            # ---- o_proj partial buffer (f32) + RS output bounce ----
            ypart = dpool.tile([L, D], dt.float32, tag="ypart")
            yqf = dpool.tile([LQ, D], dt.float32, tag="yqf")
            yqb = dpool.tile([LQ, D], dt.bfloat16, tag="yqb")

            # ---- weight slice select by partition id ----
            pid = nc.sync.partition_id()
            hgr = pid % 4

            wproj_sb = cpool.tile([128, KT, 1540], dt.bfloat16, tag="wproj")
            nc.sync.dma_start(wproj_sb[:], wproj_t[ts(hgr, 128), :, :])
            wo_sb = cpool.tile([128, NH, D], dt.bfloat16, tag="wo")
            nc.sync.dma_start(wo_sb[:], wo_t[ts(hgr, 128), :, :])
            convw_sb = cpool.tile([128, 4, 3, 4], dt.float32, tag="convw")
            nc.sync.dma_start(convw_sb[:], convw_t[ts(hgr, 128), :, :, :])
            stn_sb = cpool.tile([128, 128], dt.float32, tag="stn")
            nc.sync.dma_start(stn_sb[:], stn_t[:])
            mud_sb = cpool.tile([128, 512], dt.float32, tag="mud")
            nc.sync.dma_start(mud_sb[:], mud_t[:])
            idb_sb = cpool.tile([128, 128], dt.bfloat16, tag="idb")
            nc.sync.dma_start(idb_sb[:], idb_t[:])
            idf_sb = cpool.tile([128, 128], dt.float32, tag="idf")
            nc.sync.dma_start(idf_sb[:], idf_t[:])
            idf4_sb = cpool.tile([128, 512], dt.float32, tag="idf4")
            nc.sync.dma_start(idf4_sb[:], idf4_t[:])
            oc_sb = cpool.tile([128, 1], dt.bfloat16, tag="onesc")
            nc.sync.dma_start(oc_sb[:], oc_t[:])
            or_sb = cpool.tile([1, 128], dt.bfloat16, tag="onesr")
            nc.sync.dma_start(or_sb[:], or_t[:])

            # persistent conv halo buffers (cols 0:3 = last 3 of prev superchunk)
            xbufs = []
            for ct in range(12):
                xb = xpool.tile([128, 516], dt.bfloat16, tag=f"xb{ct}")
                nc.gpsimd.memset(xb[:, 0:4], 0.0)
                xbufs.append(xb)

            s_sb = [None] * NH
            s_fp = None
            tt_gr = None

            for sc in range(NSC):
                ht_sb = htpool.tile([128, KT, SC], dt.bfloat16, tag="ht")
                nc.sync.dma_start(
                    ht_sb[:], hTg_r[sc // 2, :, :, ds((sc % 2) * 512, SC)])

                # ---- qkv + beta projection ----
                cq = []     # conv+silu outputs (q0..3, k0..3, v0..3)
                brow = smpool.tile([4, SC], dt.float32, tag="brow")
                for ct in range(13):
                    ps = psw.tile([128, SC], dt.float32, tag="w")
                    m = 128 if ct < 12 else 4
                    for kt in range(KT):
                        nc.tensor.matmul(
                            ps[0:m, :],
                            wproj_sb[:, kt, ds(ct * 128, m)],
                            ht_sb[:, kt, :],
                            start=(kt == 0), stop=(kt == KT - 1),
                        )
                    if ct < 12:
                        xb = xbufs[ct]
                        nc.scalar.copy(xb[:, 4:4 + SC], ps[:])
                        # conv: y[t] = sum_i x[t-3+i]*w_i ; x col offset 4+t-3+i
                        w = convw_sb[:, ct % 4, ct // 4, :]
                        cqt = cqpool.tile([128, SC], dt.bfloat16, tag=f"cq{ct}")
                        tmp = cvtpool.tile([128, SC], dt.bfloat16, tag="cvt")
                        nc.vector.tensor_scalar(
                            tmp[:], xb[:, 1:1 + SC], w[:, 0:1], None, OP.mult)
                        nc.vector.scalar_tensor_tensor(
                            tmp[:], xb[:, 2:2 + SC], w[:, 1:2], tmp[:],
                            OP.mult, OP.add)
                        nc.vector.scalar_tensor_tensor(
                            tmp[:], xb[:, 3:3 + SC], w[:, 2:3], tmp[:],
                            OP.mult, OP.add)
                        nc.vector.scalar_tensor_tensor(
                            tmp[:], xb[:, 4:4 + SC], w[:, 3:4], tmp[:],
                            OP.mult, OP.add)
                        nc.scalar.activation(cqt[:], tmp[:], AF.Silu)
                        # roll halo for next superchunk
                        nc.vector.tensor_copy(xb[:, 1:4], xb[:, 1 + SC:4 + SC])
                        cq.append(cqt)
                    else:
                        nc.scalar.activation(brow[:], ps[0:4, :], AF.Sigmoid)

                # ---- l2 norm for q,k tiles (ct 0..7) ----
                qn = []
                for ct in range(8):
                    x = cq[ct]
                    q2 = qnpool.tile([128, SC], dt.bfloat16, tag="q2")
                    nc.vector.tensor_tensor(q2[:], x[:], x[:], OP.mult)
                    pssq = psm.tile([128, SC], dt.float32, tag="m")
                    pss1 = pssq[0:1, :]
                    nc.tensor.matmul(pss1, oc_sb[:], q2[:], start=True, stop=True)
                    lg = smpool.tile([1, SC], dt.float32, tag="lg")
                    nc.scalar.activation(lg[:], pss1, AF.Ln, bias=1e-12)
                    rr = smpool.tile([1, SC], dt.bfloat16, tag="rr")
                    nc.scalar.activation(rr[:], lg[:], AF.Exp, scale=-0.5)
                    psb = psm.tile([128, SC], dt.float32, tag="m")
                    nc.tensor.matmul(psb[:], or_sb[:], rr[:], start=True, stop=True)
                    qt = qnpool.tile([128, SC], dt.bfloat16, tag=f"qn{ct}")
                    nc.vector.tensor_tensor(qt[:], x[:], psb[:], OP.mult)
                    qn.append(qt)

                # ---- per chunk ----
                for c in range(NCH):
                    gc = sc * NCH + c
                    csl = ds(c * C, C)

                    # beta column [128,4] for this chunk (+negated)
                    psbt4 = psm.tile([128, SC], dt.float32, tag="m")
                    psbt = psbt4[:, 0:4]
                    nc.tensor.transpose(psbt, brow[:, csl], idf_sb[0:4, 0:4])
                    bT = smpool.tile([128, 4], dt.float32, tag="bT")
                    nc.scalar.copy(bT[:], psbt)
                    nbT = smpool.tile([128, 4], dt.float32, tag="nbT")
                    nc.scalar.mul(nbT[:], psbt, -1.0)

                    # ---- T-build (4 heads batched per psum bank) ----
                    def hsl(h):
                        return ds(h * 128, 128)

                    psG = psw.tile([128, 512], dt.float32, tag="w")
                    for h in range(NH):
                        nc.tensor.matmul(psG[:, hsl(h)], qn[4 + h][:, csl],
                                         qn[4 + h][:, csl], start=True, stop=True)
                    nA = tbpool.tile([128, 512], dt.bfloat16, tag="nA")
                    for h in range(NH):
                        nc.vector.scalar_tensor_tensor(
                            nA[:, hsl(h)], psG[:, hsl(h)], bT[:, h:h + 1],
                            stn_sb[:], OP.mult, OP.mult)
                    psT = pst.tile([128, 512], dt.bfloat16, tag="t")
                    for h in range(NH):
                        nc.tensor.transpose(psT[:, hsl(h)], nA[:, hsl(h)], idb_sb[:])
                    nAt = tbpool.tile([128, 512], dt.bfloat16, tag="nAt")
                    nc.scalar.copy(nAt[:], psT[:])

                    pows = []   # [(A2,At2),(A4,At4),(A8,At8)]
                    lhs_lo, rhs_lo = nA, nAt
                    for lvl in range(3):
                        psq = psw.tile([128, 512], dt.float32, tag="w")
                        for h in range(NH):
                            nc.tensor.matmul(psq[:, hsl(h)], lhs_lo[:, hsl(h)],
                                             rhs_lo[:, hsl(h)], start=True, stop=True)
                        At_k = tbpool.tile([128, 512], dt.bfloat16, tag=f"At{lvl}")
                        if lvl % 2 == 0:
                            nc.vector.tensor_copy(At_k[:], psq[:])
                        else:
                            nc.scalar.copy(At_k[:], psq[:])
                        psq2 = pst.tile([128, 512], dt.bfloat16, tag="t")
                        for h in range(NH):
                            nc.tensor.transpose(psq2[:, hsl(h)], At_k[:, hsl(h)],
                                                idb_sb[:])
                        A_k = tbpool.tile([128, 512], dt.bfloat16, tag=f"A{lvl}")
                        if lvl % 2 == 0:
                            nc.scalar.copy(A_k[:], psq2[:])
                        else:
                            nc.vector.tensor_copy(A_k[:], psq2[:])
                        pows.append((A_k, At_k))
                        lhs_lo, rhs_lo = A_k, At_k

                    # At16 into psum; R0 = I + At16 (add identity in drain)
                    psP = psw.tile([128, 512], dt.float32, tag="w")
                    A8, At8 = pows[2]
                    for h in range(NH):
                        nc.tensor.matmul(psP[:, hsl(h)], A8[:, hsl(h)],
                                         At8[:, hsl(h)], start=True, stop=True)
                    R = tbpool.tile([128, 512], dt.bfloat16, tag="R0")
                    nc.vector.tensor_tensor(R[:], psP[:], idf4_sb[:], OP.add)
                    # product chain: R_new = Ak^T @ R + R  (add prev R in drain)
                    chain = [pows[2][0], pows[1][0], pows[0][0], nA]
                    for ci, Ak in enumerate(chain):
                        psQ = psw.tile([128, 512], dt.float32, tag="w")
                        for h in range(NH):
                            nc.tensor.matmul(psQ[:, hsl(h)], Ak[:, hsl(h)],
                                             R[:, hsl(h)], start=True, stop=True)
                        if ci < 3:
                            Rn = tbpool.tile([128, 512], dt.bfloat16, tag=f"R{ci + 1}")
                            nc.vector.tensor_tensor(Rn[:], psQ[:], R[:], OP.add)
                            R = Rn
                        else:
                            tt_gr = ttpool.tile([128, 512], dt.bfloat16, tag="Tt")
                            nc.vector.tensor_tensor(tt_gr[:], psQ[:], R[:], OP.add)

                    # ---- recurrence ----
                    # vbTM = beta * v^T  (time-major)
                    psV = pst.tile([128, 512], dt.bfloat16, tag="t")
                    for h in range(NH):
                        nc.tensor.transpose(psV[:, hsl(h)], cq[8 + h][:, csl],
                                            idb_sb[:])
                    vbtm = pbpool.tile([128, 512], dt.bfloat16, tag="vbtm")
                    for h in range(NH):
                        nc.vector.tensor_scalar(vbtm[:, hsl(h)], psV[:, hsl(h)],
                                                bT[:, h:h + 1], None, OP.mult)

                    # r = vb - beta*(k S)
                    if gc > 0:
                        psR = psw.tile([128, 512], dt.float32, tag="w")
                        for h in range(NH):
                            nc.tensor.matmul(psR[:, hsl(h)], qn[4 + h][:, csl],
                                             s_sb[h], start=True, stop=True)
                        rv = pbpool.tile([128, 512], dt.bfloat16, tag="rv")
                        for h in range(NH):
                            nc.vector.scalar_tensor_tensor(
                                rv[:, hsl(h)], psR[:, hsl(h)], nbT[:, h:h + 1],
                                vbtm[:, hsl(h)], OP.mult, OP.add)
                    else:
                        rv = vbtm

                    # vnew = T r
                    psVN = psw.tile([128, 512], dt.float32, tag="w")
                    for h in range(NH):
                        nc.tensor.matmul(psVN[:, hsl(h)], tt_gr[:, hsl(h)],
                                         rv[:, hsl(h)], start=True, stop=True)
                    vn = pbpool.tile([128, 512], dt.bfloat16, tag="vn")
                    nc.scalar.copy(vn[:], psVN[:])

                    # attnT = mask(k^T q)
                    psA = psw.tile([128, 512], dt.float32, tag="w")
                    for h in range(NH):
                        nc.tensor.matmul(psA[:, hsl(h)], qn[4 + h][:, csl],
                                         qn[h][:, csl], start=True, stop=True)
                    at = pbpool.tile([128, 512], dt.bfloat16, tag="at")
                    nc.vector.tensor_tensor(at[:], psA[:], mud_sb[:], OP.mult)

                    # o = q S + attn vnew
                    psO = psw.tile([128, 512], dt.float32, tag="w")
                    for h in range(NH):
                        if gc > 0:
                            nc.tensor.matmul(psO[:, hsl(h)], qn[h][:, csl],
                                             s_sb[h], start=True, stop=False)
                        nc.tensor.matmul(psO[:, hsl(h)], at[:, hsl(h)],
                                         vn[:, hsl(h)], start=(gc == 0), stop=True)

                    # kTM (time-major k) and S += k^T vnew
                    psK = pst.tile([128, 512], dt.bfloat16, tag="t")
                    for h in range(NH):
                        nc.tensor.transpose(psK[:, hsl(h)], qn[4 + h][:, csl],
                                            idb_sb[:])
                    ktm = pbpool.tile([128, 512], dt.bfloat16, tag="ktm")
                    nc.scalar.copy(ktm[:], psK[:])
                    psS = psw.tile([128, 512], dt.float32, tag="w")
                    for h in range(NH):
                        nc.tensor.matmul(psS[:, hsl(h)], ktm[:, hsl(h)],
                                         vn[:, hsl(h)], start=True, stop=True)
                    s_new = spool.tile([128, 512], dt.float32, tag="sf")
                    if gc == 0:
                        nc.vector.tensor_scalar(s_new[:], psS[:], 1.0, None, OP.mult)
                    else:
                        nc.vector.tensor_tensor(s_new[:], psS[:], s_fp[:], OP.add)
                    s_fp = s_new
                    s4 = spool.tile([128, 512], dt.bfloat16, tag="s4")
                    nc.scalar.copy(s4[:], s_new[:])
                    for h in range(NH):
                        s_sb[h] = s4[:, hsl(h)]

                    # ---- RMSNorm + transpose + o_proj ----
                    o4 = pbpool.tile([128, 512], dt.float32, tag="o4")
                    nc.vector.tensor_scalar(o4[:], psO[:], 1.0, None, OP.mult)
                    ss4 = smpool.tile([128, 4], dt.float32, tag="ss4")
                    scr = pbpool.tile([128, 512], dt.bfloat16, tag="scr")
                    for h in range(NH):
                        nc.scalar.activation(scr[:, hsl(h)], o4[:, hsl(h)],
                                             AF.Square, accum_out=ss4[:, h:h + 1])
                    sq4 = smpool.tile([128, 4], dt.float32, tag="sq4")
                    nc.scalar.activation(sq4[:], ss4[:], AF.Sqrt,
                                         bias=1e-5, scale=1.0 / 128.0)
                    rr4 = smpool.tile([128, 4], dt.float32, tag="rr4")
                    nc.vector.reciprocal(rr4[:], sq4[:])
                    on4 = pbpool.tile([128, 512], dt.bfloat16, tag="on4")
                    for h in range(NH):
                        nc.vector.tensor_scalar(on4[:, hsl(h)], o4[:, hsl(h)],
                                                rr4[:, h:h + 1], None, OP.mult)
                    psOT = pst.tile([128, 512], dt.bfloat16, tag="t")
                    for h in range(NH):
                        nc.tensor.transpose(psOT[:, hsl(h)], on4[:, hsl(h)],
                                            idb_sb[:])
                    ot = pbpool.tile([128, 512], dt.bfloat16, tag="ot")
                    nc.scalar.copy(ot[:], psOT[:])

                    for nt in range(4):
                        psyt = psy.tile([128, 512], dt.float32, tag="y")
                        for h in range(NH):
                            nc.tensor.matmul(psyt[:], ot[:, hsl(h)],
                                             wo_sb[:, h, ds(nt * 512, 512)],
                                             start=(h == 0), stop=(h == NH - 1))
                        y4 = pbpool.tile([128, 512], dt.float32, tag="y4")
                        if nt % 2 == 0:
                            nc.scalar.copy(y4[:], psyt[:])
                        else:
                            nc.vector.tensor_copy(y4[:], psyt[:])
                        nc.sync.dma_start(
                            ypart[ds(gc * 128, 128), ds(nt * 512, 512)], y4[:])

            # ---- sum partials across the 4 head-group cores; keep L-quarter ----
            nc.gpsimd.collective_compute(
                "ReduceScatter", mybir.AluOpType.add,
                replica_groups=groups,
                ins=[ypart[:].opt()], outs=[yqf[:].opt()],
            )
            # cast f32 -> bf16 on device before download (reuse o4/on4 slots)
            for rb in range(LQ // 128):
                for ntc in range(4):
                    yf_sb = pbpool.tile([128, 512], dt.float32, tag="o4")
                    nc.sync.dma_start(
                        yf_sb[:], yqf[ds(rb * 128, 128), ds(ntc * 512, 512)])
                    yb_sb = pbpool.tile([128, 512], dt.bfloat16, tag="on4")
                    if (rb + ntc) % 2 == 0:
                        nc.scalar.copy(yb_sb[:], yf_sb[:])
                    else:
                        nc.vector.tensor_copy(yb_sb[:], yf_sb[:])
                    nc.sync.dma_start(
                        yqb[ds(rb * 128, 128), ds(ntc * 512, 512)], yb_sb[:])
            nc.gpsimd.dma_start(yq[:], yqb[:])
    nc.compile()
    return nc


_NC_CACHE = None
_WKEY = None
_RUNNER = None


def _make_runner(nc):
    """Cached jitted SPMD executor.

    run_bass_kernel_spmd rebuilds jit(shard_map(_body)) on every call, which
    re-lowers, reloads the executable, and re-uploads host-side zero output
    buffers each run.  Build it once; make the donated zero output buffers on
    device (no H2D) each call.
    """
    import jax
    import jax.numpy as jnp
    from jax.sharding import Mesh, PartitionSpec, NamedSharding
    from jax.experimental.shard_map import shard_map
    from concourse import bass2jax, mybir

    bass2jax.install_neuronx_cc_hook()

    n_cores = 8
    partition_name = nc.partition_id_tensor.name if nc.partition_id_tensor else None
    in_names, out_names, out_avals = [], [], []
    for alloc in nc.m.functions[0].allocations:
        if not isinstance(alloc, mybir.MemoryLocationSet):
            continue
        name = alloc.memorylocations[0].name
        if alloc.kind == "ExternalInput":
            if name != partition_name:
                in_names.append(name)
        elif alloc.kind == "ExternalOutput":
            shape = tuple(alloc.tensor_shape)
            dtype = mybir.dt.np(alloc.dtype)
            out_names.append(name)
            out_avals.append(jax.core.ShapedArray(shape, dtype))
    n_params, n_outs = len(in_names), len(out_names)
    all_in = list(in_names) + list(out_names)
    if partition_name is not None:
        all_in.append(partition_name)

    def _body(*args):
        operands = list(args)
        if partition_name is not None:
            operands.append(bass2jax.partition_id_tensor())
        outs = bass2jax._bass_exec_p.bind(
            *operands,
            out_avals=tuple(out_avals),
            in_names=tuple(all_in),
            out_names=tuple(out_names),
            lowering_input_output_aliases=(),
            sim_require_finite=True,
            sim_require_nnan=True,
            nc=nc,
        )
        return tuple(outs)

    devices = jax.devices()[:n_cores]
    mesh = Mesh(np.array(devices), ("core",))
    in_specs = (PartitionSpec("core"),) * (n_params + n_outs)
    out_specs = (PartitionSpec("core"),) * n_outs
    donate = tuple(range(n_params, n_params + n_outs))
    sharded = jax.jit(
        shard_map(_body, mesh=mesh, in_specs=in_specs, out_specs=out_specs,
                  check_rep=False),
        donate_argnums=donate, keep_unused=True,
    )

    zshard = tuple(NamedSharding(mesh, PartitionSpec("core")) for _ in range(n_outs))
    zs_shapes = [(n_cores * a.shape[0], *a.shape[1:]) for a in out_avals]
    zs_dtypes = [a.dtype for a in out_avals]

    @jax.jit
    def _zeros():
        return tuple(jnp.zeros(s, d) for s, d in zip(zs_shapes, zs_dtypes))

    _zeros_sharded = jax.jit(_zeros, out_shardings=zshard) if n_outs else _zeros

    def run(in_maps):
        per_core = [[np.asarray(m[name]) for name in in_names] for m in in_maps]
        concat_in = [
            np.concatenate([per_core[c][i] for c in range(n_cores)], axis=0)
            for i in range(n_params)
        ]
        zouts = _zeros_sharded()
        out_arrs = sharded(*concat_in, *zouts)
        return [
            {name: np.asarray(out_arrs[i]).reshape(n_cores, *out_avals[i].shape)[c]
             for i, name in enumerate(out_names)}
            for c in range(n_cores)
        ]

    return run


def kernel(hidden_states, w_cattn, wq_conv, wk_conv, wv_conv, w_beta,
           o_norm_w, w_o):
    global _NC_CACHE, _WKEY, _RUNNER, _last_in_maps, _LAST_RES

    hidden_states = np.asarray(hidden_states, np.float32)
    w_cattn = np.asarray(w_cattn, np.float32)
    w_beta = np.asarray(w_beta, np.float32)
    w_o = np.asarray(w_o, np.float32)
    o_norm_w = np.asarray(o_norm_w, np.float32)
    convs = [np.asarray(x, np.float32) for x in (wq_conv, wk_conv, wv_conv)]

    wkey = (float(w_cattn[0, 0]), float(w_o[0, 0]), float(w_beta[0, 0]),
            float(convs[0][0, 0]), float(o_norm_w[0]),
            float(w_cattn.sum()), float(w_o.sum()))
    if _NC_CACHE is None or _WKEY != wkey:
        wd = _prep_weights(w_cattn, w_beta, w_o, o_norm_w, convs)
        _NC_CACHE = _build_nc(wd)
        _WKEY = wkey
        _RUNNER = _make_runner(_NC_CACHE)

    # per-core hidden L-quarter: core b*4+hg gets hT[b][:, hg*1024:(hg+1)*1024]
    in_maps = []
    for core in range(8):
        b, hgq = divmod(core, 4)
        hslab = hidden_states[b, hgq * LQ:(hgq + 1) * LQ, :]   # [1024, 2048]
        in_maps.append({"hq": np.ascontiguousarray(hslab.T).astype(BF)})
    _last_in_maps = in_maps

    results = _RUNNER(in_maps)
    _LAST_RES = results
    out = np.empty((B, L, D), np.float32)
    for core in range(8):
        b, hgq = divmod(core, 4)
        out[b, hgq * LQ:(hgq + 1) * LQ, :] = results[core]["yq"]
    return out


if __name__ == "__main__":
    rng = np.random.default_rng(0)
    inputs = {
        "hidden_states": rng.standard_normal((B, L, D), dtype=np.float32),
        "w_cattn": rng.standard_normal((D, 3 * D), dtype=np.float32) * 0.02,
        "wq_conv": rng.standard_normal((D, 4), dtype=np.float32) * 0.3,
        "wk_conv": rng.standard_normal((D, 4), dtype=np.float32) * 0.3,
        "wv_conv": rng.standard_normal((D, 4), dtype=np.float32) * 0.3,
        "w_beta": rng.standard_normal((D, H), dtype=np.float32) * 0.02,
        "o_norm_w": np.ones((DK,), np.float32),
        "w_o": rng.standard_normal((D, D), dtype=np.float32) * 0.02,
    }
    out = kernel(**inputs)
    print("out", out.shape, out.dtype, np.abs(out).max())


# revision 10
# speedup vs baseline: 5.9222x; 1.0365x over previous
"""DeltaNet fused kernel for 8 Trainium2 NeuronCores.

Sharding: core = b*4 + hg  (b in {0,1} batches, hg in {0..3} head-groups of 4
heads).  Each core computes its 4 heads end-to-end (qkv proj + conv + silu +
l2norm + chunked delta rule + RMSNorm + o_proj rows).

I/O strategy (the axon tunnel moves ~30-40 MB/s, so bytes over the wire
dominate wall time):
  - All weights + constants are baked into the NEFF as inline Const DRAM
    tensors (uploaded once at model load, not per execution).  Per-core
    weight slices are selected at runtime with a partition-id dynamic DMA.
  - hidden_states is uploaded L-quarter-sharded: core b*4+hg uploads only
    hT[b][:, hg*1024:(hg+1)*1024] bf16 (4.2 MB/core), then an on-device
    AllGather over groups [[0..3],[4..7]] rebuilds the full hT per batch.
  - o_proj partials are ReduceScatter-summed (bf16) over the same groups,
    so each core downloads only its fully-summed L-quarter [1024, 2048].

Chunked delta rule (chunk C=128): per chunk
    G = k k^T;  A = strict_tril(diag(beta) G);  T = (I+A)^{-1}
    r = beta*(v - k S);  vnew = T r;  o = q S + tril(q k^T) vnew;  S += k^T vnew
T^{-1} via nilpotent doubling: (I+A)^{-1} = (I-A)(I+A^2)(I+A^4)(I+A^8)(I+A^16)

All matmuls bf16 inputs with fp32 PSUM accumulation.
"""

import numpy as np
import ml_dtypes

B, L, D, H, DK = 2, 4096, 2048, 16, 128
NH = 4            # heads per core
C = 128           # chunk
SC = 512          # superchunk (4 chunks)
NSC = L // SC     # 8
NCH = SC // C     # 4
KT = D // 128     # 16 k-tiles
LQ = L // 4       # 1024, L-quarter per core
BF = ml_dtypes.bfloat16


def _prep_weights(w_cattn, w_beta, w_o, o_norm_w, convs):
    """Host-side weight layout prep -> dict of inline-tensor arrays."""
    wp_all, wo_all, cw_all = [], [], []
    for hg in range(4):
        cs = slice(hg * 512, hg * 512 + 512)
        wq = w_cattn[:, 0 * D:][:, cs]
        wk = w_cattn[:, 1 * D:][:, cs]
        wv = w_cattn[:, 2 * D:][:, cs]
        wb = w_beta[:, hg * 4:hg * 4 + 4]
        wp = np.concatenate([wq, wk, wv, wb], axis=1)         # [2048, 1540]
        wp = wp.reshape(KT, 128, 1540).transpose(1, 0, 2)     # [128, 16, 1540]
        wp_all.append(wp)
        wos = (w_o[cs, :] * np.tile(o_norm_w, 4)[:, None])    # [512, 2048]
        wos = wos.reshape(NH, 128, D).transpose(1, 0, 2)      # [128, 4, 2048]
        wo_all.append(wos)
        cw = np.stack([w[cs].reshape(NH, 128, 4).transpose(1, 0, 2)
                       for w in convs], axis=2)               # [128, 4, 3, 4]
        cw_all.append(cw)
    stril = np.tril(np.ones((128, 128), np.float32), -1)
    return {
        "wproj": np.concatenate(wp_all, 0).astype(BF),        # [512, 16, 1540]
        "wo": np.concatenate(wo_all, 0).astype(BF),           # [512, 4, 2048]
        "convw": np.concatenate(cw_all, 0).astype(np.float32),  # [512, 4, 3, 4]
        "strilneg": -stril,
        "maskud4": np.tile(np.triu(np.ones((128, 128), np.float32), 0), (1, 4)),
        "identbf": np.eye(128, dtype=BF),
        "identfp": np.eye(128, dtype=np.float32),
        "identfp4": np.tile(np.eye(128, dtype=np.float32), (1, 4)),
        "ones_col": np.ones((128, 1), BF),
        "ones_row": np.ones((1, 128), BF),
    }


def _build_nc(wd):
    import concourse.bacc as bacc
    import concourse.tile as tile
    import concourse.mybir as mybir
    from concourse.bass import ds, ts

    dt = mybir.dt
    AF = mybir.ActivationFunctionType
    OP = mybir.AluOpType

    nc = bacc.Bacc("TRN2", target_bir_lowering=False)

    # register const APs needed by activation bias args
    for val in (1e-12, 1e-5):
        t = nc.alloc_sbuf_tensor(f"const-f32-{val}", [128, 1], dt.float32)
        nc.gpsimd.memset(t.ap(), val)
        nc.const_aps.aps[(dt.float32, val)] = t.ap()
    nc.all_engine_barrier()

    # ---- per-run I/O ----
    hq = nc.dram_tensor("hq", [D, LQ], dt.bfloat16, kind="ExternalInput")
    yq = nc.dram_tensor("yq", [LQ, D], dt.bfloat16, kind="ExternalOutput")

    # ---- NEFF-resident weights/constants ----
    wproj_t = nc.inline_tensor(wd["wproj"], name="wproj")     # [512,16,1540]
    wo_t = nc.inline_tensor(wd["wo"], name="wo")              # [512,4,2048]
    convw_t = nc.inline_tensor(wd["convw"], name="convw")     # [512,4,3,4]
    stn_t = nc.inline_tensor(wd["strilneg"], name="strilneg")
    mud_t = nc.inline_tensor(wd["maskud4"], name="maskud4")
    idb_t = nc.inline_tensor(wd["identbf"], name="identbf")
    idf_t = nc.inline_tensor(wd["identfp"], name="identfp")
    idf4_t = nc.inline_tensor(wd["identfp4"], name="identfp4")
    oc_t = nc.inline_tensor(wd["ones_col"], name="ones_col")
    or_t = nc.inline_tensor(wd["ones_row"], name="ones_row")

    groups = [[0, 1, 2, 3], [4, 5, 6, 7]]

    with tile.TileContext(nc) as tc:
        with (
            tc.tile_pool(name="const", bufs=1) as cpool,
            tc.tile_pool(name="xbuf", bufs=1) as xpool,
            tc.tile_pool(name="ht", bufs=2) as htpool,
            tc.tile_pool(name="cq", bufs=1) as cqpool,
            tc.tile_pool(name="cvt", bufs=4) as cvtpool,
            tc.tile_pool(name="qn", bufs=1) as qnpool,
            tc.tile_pool(name="tb", bufs=1) as tbpool,
            tc.tile_pool(name="tt", bufs=2) as ttpool,
            tc.tile_pool(name="pb", bufs=2) as pbpool,
            tc.tile_pool(name="ssb", bufs=2) as spool,
            tc.tile_pool(name="small", bufs=2) as smpool,
            tc.tile_pool(name="psw", bufs=3, space="PSUM") as psw,
            tc.tile_pool(name="pst", bufs=2, space="PSUM") as pst,
            tc.tile_pool(name="psy", bufs=2, space="PSUM") as psy,
            tc.tile_pool(name="psm", bufs=1, space="PSUM") as psm,
            tc.tile_pool(name="dram", bufs=1, space="DRAM") as dpool,
        ):
            # ---- hidden AllGather: quarter -> full hT of this core's batch ----
            hbounce = dpool.tile([D, LQ], dt.bfloat16, tag="hbounce")
            hTg = dpool.tile([4, D, LQ], dt.bfloat16, tag="hTg")
            nc.gpsimd.dma_start(hbounce[:], hq[:])
            nc.gpsimd.collective_compute(
                "AllGather", mybir.AluOpType.bypass,
                replica_groups=groups,
                ins=[hbounce[:].opt()], outs=[hTg[:].opt()],
            )
            # [4, (kt p), l] -> [4, p, kt, l] view for superchunk loads
            hTg_r = hTg.rearrange("bk (kt p) l -> bk p kt l", p=128)

            # ---- o_proj partial buffer (f32) + RS output bounce ----
            ypart = dpool.tile([L, D], dt.float32, tag="ypart")
            yqf = dpool.tile([LQ, D], dt.float32, tag="yqf")
            yqb = dpool.tile([LQ, D], dt.bfloat16, tag="yqb")

            # ---- weight slice select by partition id ----
            pid = nc.sync.partition_id()
            hgr = pid % 4

            wproj_sb = cpool.tile([128, KT, 1540], dt.bfloat16, tag="wproj")
            nc.sync.dma_start(wproj_sb[:], wproj_t[ts(hgr, 128), :, :])
            wo_sb = cpool.tile([128, NH, D], dt.bfloat16, tag="wo")
            nc.sync.dma_start(wo_sb[:], wo_t[ts(hgr, 128), :, :])
            convw_sb = cpool.tile([128, 4, 3, 4], dt.float32, tag="convw")
            nc.sync.dma_start(convw_sb[:], convw_t[ts(hgr, 128), :, :, :])
            stn_sb = cpool.tile([128, 128], dt.float32, tag="stn")
            nc.sync.dma_start(stn_sb[:], stn_t[:])
            mud_sb = cpool.tile([128, 512], dt.float32, tag="mud")
            nc.sync.dma_start(mud_sb[:], mud_t[:])
            idb_sb = cpool.tile([128, 128], dt.bfloat16, tag="idb")
            nc.sync.dma_start(idb_sb[:], idb_t[:])
            idf_sb = cpool.tile([128, 128], dt.float32, tag="idf")
            nc.sync.dma_start(idf_sb[:], idf_t[:])
            idf4_sb = cpool.tile([128, 512], dt.float32, tag="idf4")
            nc.sync.dma_start(idf4_sb[:], idf4_t[:])
            oc_sb = cpool.tile([128, 1], dt.bfloat16, tag="onesc")
            nc.sync.dma_start(oc_sb[:], oc_t[:])
            or_sb = cpool.tile([1, 128], dt.bfloat16, tag="onesr")
            nc.sync.dma_start(or_sb[:], or_t[:])

            # persistent conv halo buffers (cols 0:3 = last 3 of prev superchunk)
            xbufs = []
            for ct in range(12):
                xb = xpool.tile([128, 516], dt.bfloat16, tag=f"xb{ct}")
                nc.gpsimd.memset(xb[:, 0:4], 0.0)
                xbufs.append(xb)

            s_sb = [None] * NH
            s_fp = None
            tt_gr = None

            for sc in range(NSC):
                ht_sb = htpool.tile([128, KT, SC], dt.bfloat16, tag="ht")
                nc.sync.dma_start(
                    ht_sb[:], hTg_r[sc // 2, :, :, ds((sc % 2) * 512, SC)])

                # ---- qkv + beta projection ----
                cq = []     # conv+silu outputs (q0..3, k0..3, v0..3)
                brow = smpool.tile([4, SC], dt.float32, tag="brow")
                for ct in range(13):
                    ps = psw.tile([128, SC], dt.float32, tag="w")
                    m = 128 if ct < 12 else 4
                    for kt in range(KT):
                        nc.tensor.matmul(
                            ps[0:m, :],
                            wproj_sb[:, kt, ds(ct * 128, m)],
                            ht_sb[:, kt, :],
                            start=(kt == 0), stop=(kt == KT - 1),
                        )
                    if ct < 12:
                        xb = xbufs[ct]
                        nc.scalar.copy(xb[:, 4:4 + SC], ps[:])
                        # conv: y[t] = sum_i x[t-3+i]*w_i ; x col offset 4+t-3+i
                        w = convw_sb[:, ct % 4, ct // 4, :]
                        cqt = cqpool.tile([128, SC], dt.bfloat16, tag=f"cq{ct}")
                        tmp = cvtpool.tile([128, SC], dt.bfloat16, tag="cvt")
                        nc.vector.tensor_scalar(
                            tmp[:], xb[:, 1:1 + SC], w[:, 0:1], None, OP.mult)
                        nc.vector.scalar_tensor_tensor(
                            tmp[:], xb[:, 2:2 + SC], w[:, 1:2], tmp[:],
                            OP.mult, OP.add)
                        nc.vector.scalar_tensor_tensor(
                            tmp[:], xb[:, 3:3 + SC], w[:, 2:3], tmp[:],
                            OP.mult, OP.add)
                        nc.vector.scalar_tensor_tensor(
                            tmp[:], xb[:, 4:4 + SC], w[:, 3:4], tmp[:],
                            OP.mult, OP.add)
                        nc.scalar.activation(cqt[:], tmp[:], AF.Silu)
                        # roll halo for next superchunk
                        nc.vector.tensor_copy(xb[:, 1:4], xb[:, 1 + SC:4 + SC])
                        cq.append(cqt)
                    else:
                        nc.scalar.activation(brow[:], ps[0:4, :], AF.Sigmoid)

                # ---- l2 norm for q,k tiles (ct 0..7) ----
                qn = []
                for ct in range(8):
                    x = cq[ct]
                    q2 = qnpool.tile([128, SC], dt.bfloat16, tag="q2")
                    nc.vector.tensor_tensor(q2[:], x[:], x[:], OP.mult)
                    pssq = psm.tile([128, SC], dt.float32, tag="m")
                    pss1 = pssq[0:1, :]
                    nc.tensor.matmul(pss1, oc_sb[:], q2[:], start=True, stop=True)
                    lg = smpool.tile([1, SC], dt.float32, tag="lg")
                    nc.scalar.activation(lg[:], pss1, AF.Ln, bias=1e-12)
                    rr = smpool.tile([1, SC], dt.bfloat16, tag="rr")
                    nc.scalar.activation(rr[:], lg[:], AF.Exp, scale=-0.5)
                    psb = psm.tile([128, SC], dt.float32, tag="m")
                    nc.tensor.matmul(psb[:], or_sb[:], rr[:], start=True, stop=True)
                    qt = qnpool.tile([128, SC], dt.bfloat16, tag=f"qn{ct}")
                    nc.vector.tensor_tensor(qt[:], x[:], psb[:], OP.mult)
                    qn.append(qt)

                # ---- per chunk ----
                for c in range(NCH):
                    gc = sc * NCH + c
                    csl = ds(c * C, C)

                    # beta column [128,4] for this chunk (+negated)
                    psbt4 = psm.tile([128, SC], dt.float32, tag="m")
                    psbt = psbt4[:, 0:4]
                    nc.tensor.transpose(psbt, brow[:, csl], idf_sb[0:4, 0:4])
                    bT = smpool.tile([128, 4], dt.float32, tag="bT")
                    nc.scalar.copy(bT[:], psbt)
                    nbT = smpool.tile([128, 4], dt.float32, tag="nbT")
                    nc.scalar.mul(nbT[:], psbt, -1.0)

                    # ---- T-build (4 heads batched per psum bank) ----
                    def hsl(h):
                        return ds(h * 128, 128)

                    psG = psw.tile([128, 512], dt.float32, tag="w")
                    for h in range(NH):
                        nc.tensor.matmul(psG[:, hsl(h)], qn[4 + h][:, csl],
                                         qn[4 + h][:, csl], start=True, stop=True)
                    nA = tbpool.tile([128, 512], dt.bfloat16, tag="nA")
                    for h in range(NH):
                        nc.vector.scalar_tensor_tensor(
                            nA[:, hsl(h)], psG[:, hsl(h)], bT[:, h:h + 1],
                            stn_sb[:], OP.mult, OP.mult)
                    psT = pst.tile([128, 512], dt.bfloat16, tag="t")
                    for h in range(NH):
                        nc.tensor.transpose(psT[:, hsl(h)], nA[:, hsl(h)], idb_sb[:])
                    nAt = tbpool.tile([128, 512], dt.bfloat16, tag="nAt")
                    nc.scalar.copy(nAt[:], psT[:])

                    pows = []   # [(A2,At2),(A4,At4),(A8,At8)]
                    lhs_lo, rhs_lo = nA, nAt
                    for lvl in range(3):
                        psq = psw.tile([128, 512], dt.float32, tag="w")
                        for h in range(NH):
                            nc.tensor.matmul(psq[:, hsl(h)], lhs_lo[:, hsl(h)],
                                             rhs_lo[:, hsl(h)], start=True, stop=True)
                        At_k = tbpool.tile([128, 512], dt.bfloat16, tag=f"At{lvl}")
                        if lvl % 2 == 0:
                            nc.vector.tensor_copy(At_k[:], psq[:])
                        else:
                            nc.scalar.copy(At_k[:], psq[:])
                        psq2 = pst.tile([128, 512], dt.bfloat16, tag="t")
                        for h in range(NH):
                            nc.tensor.transpose(psq2[:, hsl(h)], At_k[:, hsl(h)],
                                                idb_sb[:])
                        A_k = tbpool.tile([128, 512], dt.bfloat16, tag=f"A{lvl}")
                        if lvl % 2 == 0:
                            nc.scalar.copy(A_k[:], psq2[:])
                        else:
                            nc.vector.tensor_copy(A_k[:], psq2[:])
                        pows.append((A_k, At_k))
                        lhs_lo, rhs_lo = A_k, At_k

                    # At16 into psum; R0 = I + At16 (add identity in drain)
                    psP = psw.tile([128, 512], dt.float32, tag="w")
                    A8, At8 = pows[2]
                    for h in range(NH):
                        nc.tensor.matmul(psP[:, hsl(h)], A8[:, hsl(h)],
                                         At8[:, hsl(h)], start=True, stop=True)
                    R = tbpool.tile([128, 512], dt.bfloat16, tag="R0")
                    nc.vector.tensor_tensor(R[:], psP[:], idf4_sb[:], OP.add)
                    # product chain: R_new = Ak^T @ R + R  (add prev R in drain)
                    chain = [pows[2][0], pows[1][0], pows[0][0], nA]
                    for ci, Ak in enumerate(chain):
                        psQ = psw.tile([128, 512], dt.float32, tag="w")
                        for h in range(NH):
                            nc.tensor.matmul(psQ[:, hsl(h)], Ak[:, hsl(h)],
                                             R[:, hsl(h)], start=True, stop=True)
                        if ci < 3:
                            Rn = tbpool.tile([128, 512], dt.bfloat16, tag=f"R{ci + 1}")
                            nc.vector.tensor_tensor(Rn[:], psQ[:], R[:], OP.add)
                            R = Rn
                        else:
                            tt_gr = ttpool.tile([128, 512], dt.bfloat16, tag="Tt")
                            nc.vector.tensor_tensor(tt_gr[:], psQ[:], R[:], OP.add)

                    # ---- recurrence ----
                    # vbTM = beta * v^T  (time-major)
                    psV = pst.tile([128, 512], dt.bfloat16, tag="t")
                    for h in range(NH):
                        nc.tensor.transpose(psV[:, hsl(h)], cq[8 + h][:, csl],
                                            idb_sb[:])
                    vbtm = pbpool.tile([128, 512], dt.bfloat16, tag="vbtm")
                    for h in range(NH):
                        nc.vector.tensor_scalar(vbtm[:, hsl(h)], psV[:, hsl(h)],
                                                bT[:, h:h + 1], None, OP.mult)

                    # r = vb - beta*(k S)
                    if gc > 0:
                        psR = psw.tile([128, 512], dt.float32, tag="w")
                        for h in range(NH):
                            nc.tensor.matmul(psR[:, hsl(h)], qn[4 + h][:, csl],
                                             s_sb[h], start=True, stop=True)
                        rv = pbpool.tile([128, 512], dt.bfloat16, tag="rv")
                        for h in range(NH):
                            nc.vector.scalar_tensor_tensor(
                                rv[:, hsl(h)], psR[:, hsl(h)], nbT[:, h:h + 1],
                                vbtm[:, hsl(h)], OP.mult, OP.add)
                    else:
                        rv = vbtm

                    # vnew = T r
                    psVN = psw.tile([128, 512], dt.float32, tag="w")
                    for h in range(NH):
                        nc.tensor.matmul(psVN[:, hsl(h)], tt_gr[:, hsl(h)],
                                         rv[:, hsl(h)], start=True, stop=True)
                    vn = pbpool.tile([128, 512], dt.bfloat16, tag="vn")
                    nc.scalar.copy(vn[:], psVN[:])

                    # attnT = mask(k^T q)
                    psA = psw.tile([128, 512], dt.float32, tag="w")
                    for h in range(NH):
                        nc.tensor.matmul(psA[:, hsl(h)], qn[4 + h][:, csl],
                                         qn[h][:, csl], start=True, stop=True)
                    at = pbpool.tile([128, 512], dt.bfloat16, tag="at")
                    nc.vector.tensor_tensor(at[:], psA[:], mud_sb[:], OP.mult)

                    # o = q S + attn vnew
                    psO = psw.tile([128, 512], dt.float32, tag="w")
                    for h in range(NH):
                        if gc > 0:
                            nc.tensor.matmul(psO[:, hsl(h)], qn[h][:, csl],
                                             s_sb[h], start=True, stop=False)
                        nc.tensor.matmul(psO[:, hsl(h)], at[:, hsl(h)],
                                         vn[:, hsl(h)], start=(gc == 0), stop=True)

                    # kTM (time-major k) and S += k^T vnew
                    psK = pst.tile([128, 512], dt.bfloat16, tag="t")
                    for h in range(NH):
                        nc.tensor.transpose(psK[:, hsl(h)], qn[4 + h][:, csl],
                                            idb_sb[:])
                    ktm = pbpool.tile([128, 512], dt.bfloat16, tag="ktm")
                    nc.scalar.copy(ktm[:], psK[:])
                    psS = psw.tile([128, 512], dt.float32, tag="w")
                    for h in range(NH):
                        nc.tensor.matmul(psS[:, hsl(h)], ktm[:, hsl(h)],
                                         vn[:, hsl(h)], start=True, stop=True)
                    s_new = spool.tile([128, 512], dt.float32, tag="sf")
                    if gc == 0:
                        nc.vector.tensor_scalar(s_new[:], psS[:], 1.0, None, OP.mult)
                    else:
                        nc.vector.tensor_tensor(s_new[:], psS[:], s_fp[:], OP.add)
                    s_fp = s_new
                    s4 = spool.tile([128, 512], dt.bfloat16, tag="s4")
                    nc.scalar.copy(s4[:], s_new[:])
                    for h in range(NH):
                        s_sb[h] = s4[:, hsl(h)]

                    # ---- RMSNorm + transpose + o_proj ----
                    o4 = pbpool.tile([128, 512], dt.float32, tag="o4")
                    nc.vector.tensor_scalar(o4[:], psO[:], 1.0, None, OP.mult)
                    ss4 = smpool.tile([128, 4], dt.float32, tag="ss4")
                    scr = pbpool.tile([128, 512], dt.bfloat16, tag="scr")
                    for h in range(NH):
                        nc.scalar.activation(scr[:, hsl(h)], o4[:, hsl(h)],
                                             AF.Square, accum_out=ss4[:, h:h + 1])
                    sq4 = smpool.tile([128, 4], dt.float32, tag="sq4")
                    nc.scalar.activation(sq4[:], ss4[:], AF.Sqrt,
                                         bias=1e-5, scale=1.0 / 128.0)
                    rr4 = smpool.tile([128, 4], dt.float32, tag="rr4")
                    nc.vector.reciprocal(rr4[:], sq4[:])
                    on4 = pbpool.tile([128, 512], dt.bfloat16, tag="on4")
                    for h in range(NH):
                        nc.vector.tensor_scalar(on4[:, hsl(h)], o4[:, hsl(h)],
                                                rr4[:, h:h + 1], None, OP.mult)
                    psOT = pst.tile([128, 512], dt.bfloat16, tag="t")
                    for h in range(NH):
                        nc.tensor.transpose(psOT[:, hsl(h)], on4[:, hsl(h)],
                                            idb_sb[:])
                    ot = pbpool.tile([128, 512], dt.bfloat16, tag="ot")
                    nc.scalar.copy(ot[:], psOT[:])

                    for nt in range(4):
                        psyt = psy.tile([128, 512], dt.float32, tag="y")
                        for h in range(NH):
                            nc.tensor.matmul(psyt[:], ot[:, hsl(h)],
                                             wo_sb[:, h, ds(nt * 512, 512)],
                                             start=(h == 0), stop=(h == NH - 1))
                        y4 = pbpool.tile([128, 512], dt.float32, tag="y4")
                        if nt % 2 == 0:
                            nc.scalar.copy(y4[:], psyt[:])
                        else:
                            nc.vector.tensor_copy(y4[:], psyt[:])
                        nc.sync.dma_start(
                            ypart[ds(gc * 128, 128), ds(nt * 512, 512)], y4[:])

            # ---- sum partials across the 4 head-group cores; keep L-quarter ----
            nc.gpsimd.collective_compute(
                "ReduceScatter", mybir.AluOpType.add,
                replica_groups=groups,
                ins=[ypart[:].opt()], outs=[yqf[:].opt()],
            )
            # cast f32 -> bf16 on device before download (reuse o4/on4 slots)
            for rb in range(LQ // 128):
                for ntc in range(4):
                    yf_sb = pbpool.tile([128, 512], dt.float32, tag="o4")
                    nc.sync.dma_start(
                        yf_sb[:], yqf[ds(rb * 128, 128), ds(ntc * 512, 512)])
                    yb_sb = pbpool.tile([128, 512], dt.bfloat16, tag="on4")
                    if (rb + ntc) % 2 == 0:
                        nc.scalar.copy(yb_sb[:], yf_sb[:])
                    else:
                        nc.vector.tensor_copy(yb_sb[:], yf_sb[:])
                    nc.sync.dma_start(
                        yqb[ds(rb * 128, 128), ds(ntc * 512, 512)], yb_sb[:])
            nc.gpsimd.dma_start(yq[:], yqb[:])
    nc.compile()
    return nc


_NC_CACHE = None
_WKEY = None
_RUNNER = None


def _make_runner(nc):
    """Cached jitted SPMD executor.

    run_bass_kernel_spmd rebuilds jit(shard_map(_body)) on every call, which
    re-lowers, reloads the executable, and re-uploads host-side zero output
    buffers each run.  Build it once; make the donated zero output buffers on
    device (no H2D) each call.
    """
    import jax
    import jax.numpy as jnp
    from jax.sharding import Mesh, PartitionSpec, NamedSharding
    from jax.experimental.shard_map import shard_map
    from concourse import bass2jax, mybir

    bass2jax.install_neuronx_cc_hook()

    n_cores = 8
    partition_name = nc.partition_id_tensor.name if nc.partition_id_tensor else None
    in_names, out_names, out_avals = [], [], []
    for alloc in nc.m.functions[0].allocations:
        if not isinstance(alloc, mybir.MemoryLocationSet):
            continue
        name = alloc.memorylocations[0].name
        if alloc.kind == "ExternalInput":
            if name != partition_name:
                in_names.append(name)
        elif alloc.kind == "ExternalOutput":
            shape = tuple(alloc.tensor_shape)
            dtype = mybir.dt.np(alloc.dtype)
            out_names.append(name)
            out_avals.append(jax.core.ShapedArray(shape, dtype))
    n_params, n_outs = len(in_names), len(out_names)
    all_in = list(in_names) + list(out_names)
    if partition_name is not None:
        all_in.append(partition_name)

    def _body(*args):
        operands = list(args)
        if partition_name is not None:
            operands.append(bass2jax.partition_id_tensor())
        outs = bass2jax._bass_exec_p.bind(
            *operands,
            out_avals=tuple(out_avals),
            in_names=tuple(all_in),
            out_names=tuple(out_names),
            lowering_input_output_aliases=(),
            sim_require_finite=True,
            sim_require_nnan=True,
            nc=nc,
        )
        return tuple(outs)

    devices = jax.devices()[:n_cores]
    mesh = Mesh(np.array(devices), ("core",))
    in_specs = (PartitionSpec("core"),) * (n_params + n_outs)
    out_specs = (PartitionSpec("core"),) * n_outs
    donate = tuple(range(n_params, n_params + n_outs))
    sharded = jax.jit(
        shard_map(_body, mesh=mesh, in_specs=in_specs, out_specs=out_specs,
                  check_rep=False),
        donate_argnums=donate, keep_unused=True,
    )

    zshard = tuple(NamedSharding(mesh, PartitionSpec("core")) for _ in range(n_outs))
    zs_shapes = [(n_cores * a.shape[0], *a.shape[1:]) for a in out_avals]
    zs_dtypes = [a.dtype for a in out_avals]

    @jax.jit
    def _zeros():
        return tuple(jnp.zeros(s, d) for s, d in zip(zs_shapes, zs_dtypes))

    _zeros_sharded = jax.jit(_zeros, out_shardings=zshard) if n_outs else _zeros

    from concurrent.futures import ThreadPoolExecutor
    in_shard = NamedSharding(mesh, PartitionSpec("core"))
    _pool = ThreadPoolExecutor(n_cores)

    def run(in_maps):
        # on-device zero outputs first (overlaps with uploads)
        zouts = _zeros_sharded()
        # per-device async uploads in device order -> early cores start early
        glob_in = []
        for i, name in enumerate(in_names):
            pieces = [jax.device_put(np.asarray(in_maps[c][name]), devices[c])
                      for c in range(n_cores)]
            s0 = pieces[0].shape
            glob_in.append(jax.make_array_from_single_device_arrays(
                (n_cores * s0[0], *s0[1:]), in_shard, pieces))
        out_arrs = sharded(*glob_in, *zouts)
        # threaded per-shard fetch: early groups download while late ones run
        fetched = []
        for i in range(n_outs):
            shards = sorted(out_arrs[i].addressable_shards,
                            key=lambda s: s.index[0].start or 0)
            fetched.append(list(_pool.map(lambda s: np.asarray(s.data), shards)))
        return [
            {name: fetched[i][c] for i, name in enumerate(out_names)}
            for c in range(n_cores)
        ]

    return run


def kernel(hidden_states, w_cattn, wq_conv, wk_conv, wv_conv, w_beta,
           o_norm_w, w_o):
    global _NC_CACHE, _WKEY, _RUNNER, _last_in_maps, _LAST_RES

    hidden_states = np.asarray(hidden_states, np.float32)
    w_cattn = np.asarray(w_cattn, np.float32)
    w_beta = np.asarray(w_beta, np.float32)
    w_o = np.asarray(w_o, np.float32)
    o_norm_w = np.asarray(o_norm_w, np.float32)
    convs = [np.asarray(x, np.float32) for x in (wq_conv, wk_conv, wv_conv)]

    wkey = (float(w_cattn[0, 0]), float(w_o[0, 0]), float(w_beta[0, 0]),
            float(convs[0][0, 0]), float(o_norm_w[0]),
            float(w_cattn.sum()), float(w_o.sum()))
    if _NC_CACHE is None or _WKEY != wkey:
        wd = _prep_weights(w_cattn, w_beta, w_o, o_norm_w, convs)
        _NC_CACHE = _build_nc(wd)
        _WKEY = wkey
        _RUNNER = _make_runner(_NC_CACHE)

    # per-core hidden L-quarter: core b*4+hg gets hT[b][:, hg*1024:(hg+1)*1024]
    in_maps = []
    for core in range(8):
        b, hgq = divmod(core, 4)
        hslab = hidden_states[b, hgq * LQ:(hgq + 1) * LQ, :]   # [1024, 2048]
        in_maps.append({"hq": np.ascontiguousarray(hslab.T).astype(BF)})
    _last_in_maps = in_maps

    results = _RUNNER(in_maps)
    _LAST_RES = results
    out = np.empty((B, L, D), np.float32)
    for core in range(8):
        b, hgq = divmod(core, 4)
        out[b, hgq * LQ:(hgq + 1) * LQ, :] = results[core]["yq"]
    return out


if __name__ == "__main__":
    rng = np.random.default_rng(0)
    inputs = {
        "hidden_states": rng.standard_normal((B, L, D), dtype=np.float32),
        "w_cattn": rng.standard_normal((D, 3 * D), dtype=np.float32) * 0.02,
        "wq_conv": rng.standard_normal((D, 4), dtype=np.float32) * 0.3,
        "wk_conv": rng.standard_normal((D, 4), dtype=np.float32) * 0.3,
        "wv_conv": rng.standard_normal((D, 4), dtype=np.float32) * 0.3,
        "w_beta": rng.standard_normal((D, H), dtype=np.float32) * 0.02,
        "o_norm_w": np.ones((DK,), np.float32),
        "w_o": rng.standard_normal((D, D), dtype=np.float32) * 0.02,
    }
    out = kernel(**inputs)
    print("out", out.shape, out.dtype, np.abs(out).max())


# revision 12
# speedup vs baseline: 6.1972x; 1.0464x over previous
"""DeltaNet fused kernel for 8 Trainium2 NeuronCores.

Sharding: core = b*4 + hg  (b in {0,1} batches, hg in {0..3} head-groups of 4
heads).  Each core computes its 4 heads end-to-end (qkv proj + conv + silu +
l2norm + chunked delta rule + RMSNorm + o_proj rows).

I/O strategy (the axon tunnel moves ~30-40 MB/s, so bytes over the wire
dominate wall time):
  - All weights + constants are baked into the NEFF as inline Const DRAM
    tensors (uploaded once at model load, not per execution).  Per-core
    weight slices are selected at runtime with a partition-id dynamic DMA.
  - hidden_states is uploaded L-quarter-sharded: core b*4+hg uploads only
    hT[b][:, hg*1024:(hg+1)*1024] bf16 (4.2 MB/core), then an on-device
    AllGather over groups [[0..3],[4..7]] rebuilds the full hT per batch.
  - o_proj partials are ReduceScatter-summed (bf16) over the same groups,
    so each core downloads only its fully-summed L-quarter [1024, 2048].

Chunked delta rule (chunk C=128): per chunk
    G = k k^T;  A = strict_tril(diag(beta) G);  T = (I+A)^{-1}
    r = beta*(v - k S);  vnew = T r;  o = q S + tril(q k^T) vnew;  S += k^T vnew
T^{-1} via nilpotent doubling: (I+A)^{-1} = (I-A)(I+A^2)(I+A^4)(I+A^8)(I+A^16)

All matmuls bf16 inputs with fp32 PSUM accumulation.
"""

import numpy as np
import ml_dtypes

B, L, D, H, DK = 2, 4096, 2048, 16, 128
NH = 4            # heads per core
C = 128           # chunk
SC = 512          # superchunk (4 chunks)
NSC = L // SC     # 8
NCH = SC // C     # 4
KT = D // 128     # 16 k-tiles
LQ = L // 4       # 1024, L-quarter per core
BF = ml_dtypes.bfloat16


def _prep_weights(w_cattn, w_beta, w_o, o_norm_w, convs):
    """Host-side weight layout prep -> dict of inline-tensor arrays."""
    wp_all, wo_all, cw_all = [], [], []
    for hg in range(4):
        cs = slice(hg * 512, hg * 512 + 512)
        wq = w_cattn[:, 0 * D:][:, cs]
        wk = w_cattn[:, 1 * D:][:, cs]
        wv = w_cattn[:, 2 * D:][:, cs]
        wb = w_beta[:, hg * 4:hg * 4 + 4]
        wp = np.concatenate([wq, wk, wv, wb], axis=1)         # [2048, 1540]
        wp = wp.reshape(KT, 128, 1540).transpose(1, 0, 2)     # [128, 16, 1540]
        wp_all.append(wp)
        wos = (w_o[cs, :] * np.tile(o_norm_w, 4)[:, None])    # [512, 2048]
        wos = wos.reshape(NH, 128, D).transpose(1, 0, 2)      # [128, 4, 2048]
        wo_all.append(wos)
        cw = np.stack([w[cs].reshape(NH, 128, 4).transpose(1, 0, 2)
                       for w in convs], axis=2)               # [128, 4, 3, 4]
        cw_all.append(cw)
    stril = np.tril(np.ones((128, 128), np.float32), -1)
    return {
        "wproj": np.concatenate(wp_all, 0).astype(BF),        # [512, 16, 1540]
        "wo": np.concatenate(wo_all, 0).astype(BF),           # [512, 4, 2048]
        "convw": np.concatenate(cw_all, 0).astype(np.float32),  # [512, 4, 3, 4]
        "strilneg": -stril,
        "maskud4": np.tile(np.triu(np.ones((128, 128), np.float32), 0), (1, 4)),
        "identbf": np.eye(128, dtype=BF),
        "identfp": np.eye(128, dtype=np.float32),
        "identfp4": np.tile(np.eye(128, dtype=np.float32), (1, 4)),
        "ones_col": np.ones((128, 1), BF),
        "ones_row": np.ones((1, 128), BF),
    }


def _build_nc(wd):
    import concourse.bacc as bacc
    import concourse.tile as tile
    import concourse.mybir as mybir
    from concourse.bass import ds, ts

    dt = mybir.dt
    AF = mybir.ActivationFunctionType
    OP = mybir.AluOpType

    nc = bacc.Bacc("TRN2", target_bir_lowering=False)

    # register const APs needed by activation bias args
    for val in (1e-12, 1e-5):
        t = nc.alloc_sbuf_tensor(f"const-f32-{val}", [128, 1], dt.float32)
        nc.gpsimd.memset(t.ap(), val)
        nc.const_aps.aps[(dt.float32, val)] = t.ap()
    nc.all_engine_barrier()

    # ---- per-run I/O ----
    hq = nc.dram_tensor("hq", [D, LQ], dt.bfloat16, kind="ExternalInput")
    yq = nc.dram_tensor("yq", [LQ, D], dt.bfloat16, kind="ExternalOutput")

    # ---- NEFF-resident weights/constants ----
    wproj_t = nc.inline_tensor(wd["wproj"], name="wproj")     # [512,16,1540]
    wo_t = nc.inline_tensor(wd["wo"], name="wo")              # [512,4,2048]
    convw_t = nc.inline_tensor(wd["convw"], name="convw")     # [512,4,3,4]
    stn_t = nc.inline_tensor(wd["strilneg"], name="strilneg")
    mud_t = nc.inline_tensor(wd["maskud4"], name="maskud4")
    idb_t = nc.inline_tensor(wd["identbf"], name="identbf")
    idf_t = nc.inline_tensor(wd["identfp"], name="identfp")
    idf4_t = nc.inline_tensor(wd["identfp4"], name="identfp4")
    oc_t = nc.inline_tensor(wd["ones_col"], name="ones_col")
    or_t = nc.inline_tensor(wd["ones_row"], name="ones_row")

    groups = [[0, 1, 2, 3], [4, 5, 6, 7]]

    with tile.TileContext(nc) as tc:
        with (
            tc.tile_pool(name="const", bufs=1) as cpool,
            tc.tile_pool(name="xbuf", bufs=1) as xpool,
            tc.tile_pool(name="ht", bufs=2) as htpool,
            tc.tile_pool(name="cq", bufs=1) as cqpool,
            tc.tile_pool(name="cvt", bufs=4) as cvtpool,
            tc.tile_pool(name="qn", bufs=1) as qnpool,
            tc.tile_pool(name="tb", bufs=1) as tbpool,
            tc.tile_pool(name="tt", bufs=2) as ttpool,
            tc.tile_pool(name="pb", bufs=2) as pbpool,
            tc.tile_pool(name="ssb", bufs=2) as spool,
            tc.tile_pool(name="small", bufs=2) as smpool,
            tc.tile_pool(name="psw", bufs=3, space="PSUM") as psw,
            tc.tile_pool(name="pst", bufs=2, space="PSUM") as pst,
            tc.tile_pool(name="psy", bufs=2, space="PSUM") as psy,
            tc.tile_pool(name="psm", bufs=1, space="PSUM") as psm,
            tc.tile_pool(name="dram", bufs=1, space="DRAM") as dpool,
        ):
            # ---- hidden AllGather: quarter -> full hT of this core's batch ----
            hbounce = dpool.tile([D, LQ], dt.bfloat16, tag="hbounce")
            hTg = dpool.tile([4, D, LQ], dt.bfloat16, tag="hTg")
            nc.gpsimd.dma_start(hbounce[:], hq[:])
            nc.gpsimd.collective_compute(
                "AllGather", mybir.AluOpType.bypass,
                replica_groups=groups,
                ins=[hbounce[:].opt()], outs=[hTg[:].opt()],
            )
            # [4, (kt p), l] -> [4, p, kt, l] view for superchunk loads
            hTg_r = hTg.rearrange("bk (kt p) l -> bk p kt l", p=128)

            # ---- o_proj partial buffer (f32) + RS output bounce ----
            ypart = dpool.tile([L, D], dt.float32, tag="ypart")
            yqf = dpool.tile([LQ, D], dt.float32, tag="yqf")
            yqb = dpool.tile([LQ, D], dt.bfloat16, tag="yqb")

            # ---- weight slice select by partition id ----
            pid = nc.sync.partition_id()
            hgr = pid % 4

            wproj_sb = cpool.tile([128, KT, 1540], dt.bfloat16, tag="wproj")
            nc.sync.dma_start(wproj_sb[:], wproj_t[ts(hgr, 128), :, :])
            wo_sb = cpool.tile([128, NH, D], dt.bfloat16, tag="wo")
            nc.sync.dma_start(wo_sb[:], wo_t[ts(hgr, 128), :, :])
            convw_sb = cpool.tile([128, 4, 3, 4], dt.float32, tag="convw")
            nc.sync.dma_start(convw_sb[:], convw_t[ts(hgr, 128), :, :, :])
            stn_sb = cpool.tile([128, 128], dt.float32, tag="stn")
            nc.sync.dma_start(stn_sb[:], stn_t[:])
            mud_sb = cpool.tile([128, 512], dt.float32, tag="mud")
            nc.sync.dma_start(mud_sb[:], mud_t[:])
            idb_sb = cpool.tile([128, 128], dt.bfloat16, tag="idb")
            nc.sync.dma_start(idb_sb[:], idb_t[:])
            idf_sb = cpool.tile([128, 128], dt.float32, tag="idf")
            nc.sync.dma_start(idf_sb[:], idf_t[:])
            idf4_sb = cpool.tile([128, 512], dt.float32, tag="idf4")
            nc.sync.dma_start(idf4_sb[:], idf4_t[:])
            oc_sb = cpool.tile([128, 1], dt.bfloat16, tag="onesc")
            nc.sync.dma_start(oc_sb[:], oc_t[:])
            or_sb = cpool.tile([1, 128], dt.bfloat16, tag="onesr")
            nc.sync.dma_start(or_sb[:], or_t[:])

            # persistent conv halo buffers (cols 0:3 = last 3 of prev superchunk)
            xbufs = []
            for ct in range(12):
                xb = xpool.tile([128, 516], dt.bfloat16, tag=f"xb{ct}")
                nc.gpsimd.memset(xb[:, 0:4], 0.0)
                xbufs.append(xb)

            s_sb = [None] * NH
            s_fp = None
            tt_gr = None

            for sc in range(NSC):
                ht_sb = htpool.tile([128, KT, SC], dt.bfloat16, tag="ht")
                nc.sync.dma_start(
                    ht_sb[:], hTg_r[sc // 2, :, :, ds((sc % 2) * 512, SC)])

                # ---- qkv + beta projection ----
                cq = []     # conv+silu outputs (q0..3, k0..3, v0..3)
                brow = smpool.tile([4, SC], dt.float32, tag="brow")
                for ct in range(13):
                    ps = psw.tile([128, SC], dt.float32, tag="w")
                    m = 128 if ct < 12 else 4
                    for kt in range(KT):
                        nc.tensor.matmul(
                            ps[0:m, :],
                            wproj_sb[:, kt, ds(ct * 128, m)],
                            ht_sb[:, kt, :],
                            start=(kt == 0), stop=(kt == KT - 1),
                        )
                    if ct < 12:
                        xb = xbufs[ct]
                        nc.scalar.copy(xb[:, 4:4 + SC], ps[:])
                        # conv: y[t] = sum_i x[t-3+i]*w_i ; x col offset 4+t-3+i
                        w = convw_sb[:, ct % 4, ct // 4, :]
                        cqt = cqpool.tile([128, SC], dt.bfloat16, tag=f"cq{ct}")
                        tmp = cvtpool.tile([128, SC], dt.bfloat16, tag="cvt")
                        nc.vector.tensor_scalar(
                            tmp[:], xb[:, 1:1 + SC], w[:, 0:1], None, OP.mult)
                        nc.vector.scalar_tensor_tensor(
                            tmp[:], xb[:, 2:2 + SC], w[:, 1:2], tmp[:],
                            OP.mult, OP.add)
                        nc.vector.scalar_tensor_tensor(
                            tmp[:], xb[:, 3:3 + SC], w[:, 2:3], tmp[:],
                            OP.mult, OP.add)
                        nc.vector.scalar_tensor_tensor(
                            tmp[:], xb[:, 4:4 + SC], w[:, 3:4], tmp[:],
                            OP.mult, OP.add)
                        nc.scalar.activation(cqt[:], tmp[:], AF.Silu)
                        # roll halo for next superchunk
                        nc.vector.tensor_copy(xb[:, 1:4], xb[:, 1 + SC:4 + SC])
                        cq.append(cqt)
                    else:
                        nc.scalar.activation(brow[:], ps[0:4, :], AF.Sigmoid)

                # ---- l2 norm for q,k tiles (ct 0..7) ----
                qn = []
                for ct in range(8):
                    x = cq[ct]
                    q2 = qnpool.tile([128, SC], dt.bfloat16, tag="q2")
                    nc.vector.tensor_tensor(q2[:], x[:], x[:], OP.mult)
                    pssq = psm.tile([128, SC], dt.float32, tag="m")
                    pss1 = pssq[0:1, :]
                    nc.tensor.matmul(pss1, oc_sb[:], q2[:], start=True, stop=True)
                    lg = smpool.tile([1, SC], dt.float32, tag="lg")
                    nc.scalar.activation(lg[:], pss1, AF.Ln, bias=1e-12)
                    rr = smpool.tile([1, SC], dt.bfloat16, tag="rr")
                    nc.scalar.activation(rr[:], lg[:], AF.Exp, scale=-0.5)
                    psb = psm.tile([128, SC], dt.float32, tag="m")
                    nc.tensor.matmul(psb[:], or_sb[:], rr[:], start=True, stop=True)
                    qt = qnpool.tile([128, SC], dt.bfloat16, tag=f"qn{ct}")
                    nc.vector.tensor_tensor(qt[:], x[:], psb[:], OP.mult)
                    qn.append(qt)

                # ---- per chunk ----
                for c in range(NCH):
                    gc = sc * NCH + c
                    csl = ds(c * C, C)

                    # beta column [128,4] for this chunk (+negated)
                    psbt4 = psm.tile([128, SC], dt.float32, tag="m")
                    psbt = psbt4[:, 0:4]
                    nc.tensor.transpose(psbt, brow[:, csl], idf_sb[0:4, 0:4])
                    bT = smpool.tile([128, 4], dt.float32, tag="bT")
                    nc.scalar.copy(bT[:], psbt)
                    nbT = smpool.tile([128, 4], dt.float32, tag="nbT")
                    nc.scalar.mul(nbT[:], psbt, -1.0)

                    # ---- T-build (4 heads batched per psum bank) ----
                    def hsl(h):
                        return ds(h * 128, 128)

                    psG = psw.tile([128, 512], dt.float32, tag="w")
                    for h in range(NH):
                        nc.tensor.matmul(psG[:, hsl(h)], qn[4 + h][:, csl],
                                         qn[4 + h][:, csl], start=True, stop=True)
                    nA = tbpool.tile([128, 512], dt.bfloat16, tag="nA")
                    for h in range(NH):
                        nc.vector.scalar_tensor_tensor(
                            nA[:, hsl(h)], psG[:, hsl(h)], bT[:, h:h + 1],
                            stn_sb[:], OP.mult, OP.mult)
                    psT = pst.tile([128, 512], dt.bfloat16, tag="t")
                    for h in range(NH):
                        nc.tensor.transpose(psT[:, hsl(h)], nA[:, hsl(h)], idb_sb[:])
                    nAt = tbpool.tile([128, 512], dt.bfloat16, tag="nAt")
                    nc.scalar.copy(nAt[:], psT[:])

                    pows = []   # [(A2,At2),(A4,At4),(A8,At8)]
                    lhs_lo, rhs_lo = nA, nAt
                    for lvl in range(3):
                        psq = psw.tile([128, 512], dt.float32, tag="w")
                        for h in range(NH):
                            nc.tensor.matmul(psq[:, hsl(h)], lhs_lo[:, hsl(h)],
                                             rhs_lo[:, hsl(h)], start=True, stop=True)
                        At_k = tbpool.tile([128, 512], dt.bfloat16, tag=f"At{lvl}")
                        if lvl % 2 == 0:
                            nc.vector.tensor_copy(At_k[:], psq[:])
                        else:
                            nc.scalar.copy(At_k[:], psq[:])
                        psq2 = pst.tile([128, 512], dt.bfloat16, tag="t")
                        for h in range(NH):
                            nc.tensor.transpose(psq2[:, hsl(h)], At_k[:, hsl(h)],
                                                idb_sb[:])
                        A_k = tbpool.tile([128, 512], dt.bfloat16, tag=f"A{lvl}")
                        if lvl % 2 == 0:
                            nc.scalar.copy(A_k[:], psq2[:])
                        else:
                            nc.vector.tensor_copy(A_k[:], psq2[:])
                        pows.append((A_k, At_k))
                        lhs_lo, rhs_lo = A_k, At_k

                    # At16 into psum; R0 = I + At16 (add identity in drain)
                    psP = psw.tile([128, 512], dt.float32, tag="w")
                    A8, At8 = pows[2]
                    for h in range(NH):
                        nc.tensor.matmul(psP[:, hsl(h)], A8[:, hsl(h)],
                                         At8[:, hsl(h)], start=True, stop=True)
                    R = tbpool.tile([128, 512], dt.bfloat16, tag="R0")
                    nc.vector.tensor_tensor(R[:], psP[:], idf4_sb[:], OP.add)
                    # product chain: R_new = Ak^T @ R + R  (add prev R in drain)
                    chain = [pows[2][0], pows[1][0], pows[0][0], nA]
                    for ci, Ak in enumerate(chain):
                        psQ = psw.tile([128, 512], dt.float32, tag="w")
                        for h in range(NH):
                            nc.tensor.matmul(psQ[:, hsl(h)], Ak[:, hsl(h)],
                                             R[:, hsl(h)], start=True, stop=True)
                        if ci < 3:
                            Rn = tbpool.tile([128, 512], dt.bfloat16, tag=f"R{ci + 1}")
                            nc.vector.tensor_tensor(Rn[:], psQ[:], R[:], OP.add)
                            R = Rn
                        else:
                            tt_gr = ttpool.tile([128, 512], dt.bfloat16, tag="Tt")
                            nc.vector.tensor_tensor(tt_gr[:], psQ[:], R[:], OP.add)

                    # ---- recurrence ----
                    # vbTM = beta * v^T  (time-major)
                    psV = pst.tile([128, 512], dt.bfloat16, tag="t")
                    for h in range(NH):
                        nc.tensor.transpose(psV[:, hsl(h)], cq[8 + h][:, csl],
                                            idb_sb[:])
                    vbtm = pbpool.tile([128, 512], dt.bfloat16, tag="vbtm")
                    for h in range(NH):
                        nc.vector.tensor_scalar(vbtm[:, hsl(h)], psV[:, hsl(h)],
                                                bT[:, h:h + 1], None, OP.mult)

                    # r = vb - beta*(k S)
                    if gc > 0:
                        psR = psw.tile([128, 512], dt.float32, tag="w")
                        for h in range(NH):
                            nc.tensor.matmul(psR[:, hsl(h)], qn[4 + h][:, csl],
                                             s_sb[h], start=True, stop=True)
                        rv = pbpool.tile([128, 512], dt.bfloat16, tag="rv")
                        for h in range(NH):
                            nc.vector.scalar_tensor_tensor(
                                rv[:, hsl(h)], psR[:, hsl(h)], nbT[:, h:h + 1],
                                vbtm[:, hsl(h)], OP.mult, OP.add)
                    else:
                        rv = vbtm

                    # vnew = T r
                    psVN = psw.tile([128, 512], dt.float32, tag="w")
                    for h in range(NH):
                        nc.tensor.matmul(psVN[:, hsl(h)], tt_gr[:, hsl(h)],
                                         rv[:, hsl(h)], start=True, stop=True)
                    vn = pbpool.tile([128, 512], dt.bfloat16, tag="vn")
                    nc.scalar.copy(vn[:], psVN[:])

                    # attnT = mask(k^T q)
                    psA = psw.tile([128, 512], dt.float32, tag="w")
                    for h in range(NH):
                        nc.tensor.matmul(psA[:, hsl(h)], qn[4 + h][:, csl],
                                         qn[h][:, csl], start=True, stop=True)
                    at = pbpool.tile([128, 512], dt.bfloat16, tag="at")
                    nc.vector.tensor_tensor(at[:], psA[:], mud_sb[:], OP.mult)

                    # o = q S + attn vnew
                    psO = psw.tile([128, 512], dt.float32, tag="w")
                    for h in range(NH):
                        if gc > 0:
                            nc.tensor.matmul(psO[:, hsl(h)], qn[h][:, csl],
                                             s_sb[h], start=True, stop=False)
                        nc.tensor.matmul(psO[:, hsl(h)], at[:, hsl(h)],
                                         vn[:, hsl(h)], start=(gc == 0), stop=True)

                    # kTM (time-major k) and S += k^T vnew
                    psK = pst.tile([128, 512], dt.bfloat16, tag="t")
                    for h in range(NH):
                        nc.tensor.transpose(psK[:, hsl(h)], qn[4 + h][:, csl],
                                            idb_sb[:])
                    ktm = pbpool.tile([128, 512], dt.bfloat16, tag="ktm")
                    nc.scalar.copy(ktm[:], psK[:])
                    psS = psw.tile([128, 512], dt.float32, tag="w")
                    for h in range(NH):
                        nc.tensor.matmul(psS[:, hsl(h)], ktm[:, hsl(h)],
                                         vn[:, hsl(h)], start=True, stop=True)
                    s_new = spool.tile([128, 512], dt.float32, tag="sf")
                    if gc == 0:
                        nc.vector.tensor_scalar(s_new[:], psS[:], 1.0, None, OP.mult)
                    else:
                        nc.vector.tensor_tensor(s_new[:], psS[:], s_fp[:], OP.add)
                    s_fp = s_new
                    s4 = spool.tile([128, 512], dt.bfloat16, tag="s4")
                    nc.scalar.copy(s4[:], s_new[:])
                    for h in range(NH):
                        s_sb[h] = s4[:, hsl(h)]

                    # ---- RMSNorm + transpose + o_proj ----
                    o4 = pbpool.tile([128, 512], dt.float32, tag="o4")
                    nc.vector.tensor_scalar(o4[:], psO[:], 1.0, None, OP.mult)
                    ss4 = smpool.tile([128, 4], dt.float32, tag="ss4")
                    scr = pbpool.tile([128, 512], dt.bfloat16, tag="scr")
                    for h in range(NH):
                        nc.scalar.activation(scr[:, hsl(h)], o4[:, hsl(h)],
                                             AF.Square, accum_out=ss4[:, h:h + 1])
                    sq4 = smpool.tile([128, 4], dt.float32, tag="sq4")
                    nc.scalar.activation(sq4[:], ss4[:], AF.Sqrt,
                                         bias=1e-5, scale=1.0 / 128.0)
                    rr4 = smpool.tile([128, 4], dt.float32, tag="rr4")
                    nc.vector.reciprocal(rr4[:], sq4[:])
                    on4 = pbpool.tile([128, 512], dt.bfloat16, tag="on4")
                    for h in range(NH):
                        nc.vector.tensor_scalar(on4[:, hsl(h)], o4[:, hsl(h)],
                                                rr4[:, h:h + 1], None, OP.mult)
                    psOT = pst.tile([128, 512], dt.bfloat16, tag="t")
                    for h in range(NH):
                        nc.tensor.transpose(psOT[:, hsl(h)], on4[:, hsl(h)],
                                            idb_sb[:])
                    ot = pbpool.tile([128, 512], dt.bfloat16, tag="ot")
                    nc.scalar.copy(ot[:], psOT[:])

                    for nt in range(4):
                        psyt = psy.tile([128, 512], dt.float32, tag="y")
                        for h in range(NH):
                            nc.tensor.matmul(psyt[:], ot[:, hsl(h)],
                                             wo_sb[:, h, ds(nt * 512, 512)],
                                             start=(h == 0), stop=(h == NH - 1))
                        y4 = pbpool.tile([128, 512], dt.float32, tag="y4")
                        if nt % 2 == 0:
                            nc.scalar.copy(y4[:], psyt[:])
                        else:
                            nc.vector.tensor_copy(y4[:], psyt[:])
                        nc.sync.dma_start(
                            ypart[ds(gc * 128, 128), ds(nt * 512, 512)], y4[:])

            # ---- sum partials across the 4 head-group cores; keep L-quarter ----
            nc.gpsimd.collective_compute(
                "ReduceScatter", mybir.AluOpType.add,
                replica_groups=groups,
                ins=[ypart[:].opt()], outs=[yqf[:].opt()],
            )
            # cast f32 -> bf16 on device before download (reuse o4/on4 slots)
            for rb in range(LQ // 128):
                for ntc in range(4):
                    yf_sb = pbpool.tile([128, 512], dt.float32, tag="o4")
                    nc.sync.dma_start(
                        yf_sb[:], yqf[ds(rb * 128, 128), ds(ntc * 512, 512)])
                    yb_sb = pbpool.tile([128, 512], dt.bfloat16, tag="on4")
                    if (rb + ntc) % 2 == 0:
                        nc.scalar.copy(yb_sb[:], yf_sb[:])
                    else:
                        nc.vector.tensor_copy(yb_sb[:], yf_sb[:])
                    nc.sync.dma_start(
                        yqb[ds(rb * 128, 128), ds(ntc * 512, 512)], yb_sb[:])
            nc.gpsimd.dma_start(yq[:], yqb[:])
    nc.compile()
    return nc


_NC_CACHE = None
_WKEY = None
_RUNNER = None


def _make_runner(nc):
    """Cached per-device SPMD executor.

    run_bass_kernel_spmd rebuilds jit(shard_map(_body)) on every call, which
    re-lowers, reloads the executable, and re-uploads host-side zero output
    buffers each run.  Build it once; make the donated zero output buffers on
    device (no H2D) each call.  (Per-device independent launches were tried
    to pipeline upload/exec/download across cores, but collectives across
    separate single-device executions crash the axon worker.)
    """
    import jax
    import jax.numpy as jnp
    from jax.sharding import Mesh, PartitionSpec, NamedSharding
    from jax.experimental.shard_map import shard_map
    from concourse import bass2jax, mybir
    from concurrent.futures import ThreadPoolExecutor

    bass2jax.install_neuronx_cc_hook()

    n_cores = 8
    partition_name = nc.partition_id_tensor.name if nc.partition_id_tensor else None
    in_names, out_names, out_avals = [], [], []
    for alloc in nc.m.functions[0].allocations:
        if not isinstance(alloc, mybir.MemoryLocationSet):
            continue
        name = alloc.memorylocations[0].name
        if alloc.kind == "ExternalInput":
            if name != partition_name:
                in_names.append(name)
        elif alloc.kind == "ExternalOutput":
            shape = tuple(alloc.tensor_shape)
            dtype = mybir.dt.np(alloc.dtype)
            out_names.append(name)
            out_avals.append(jax.core.ShapedArray(shape, dtype))
    n_params, n_outs = len(in_names), len(out_names)
    all_in = list(in_names) + list(out_names)
    if partition_name is not None:
        all_in.append(partition_name)

    def _body(*args):
        operands = list(args)
        if partition_name is not None:
            operands.append(bass2jax.partition_id_tensor())
        outs = bass2jax._bass_exec_p.bind(
            *operands,
            out_avals=tuple(out_avals),
            in_names=tuple(all_in),
            out_names=tuple(out_names),
            lowering_input_output_aliases=(),
            sim_require_finite=True,
            sim_require_nnan=True,
            nc=nc,
        )
        return tuple(outs)

    devices = jax.devices()[:n_cores]
    mesh = Mesh(np.array(devices), ("core",))
    in_specs = (PartitionSpec("core"),) * (n_params + n_outs)
    out_specs = (PartitionSpec("core"),) * n_outs
    donate = tuple(range(n_params, n_params + n_outs))
    sharded = jax.jit(
        shard_map(_body, mesh=mesh, in_specs=in_specs, out_specs=out_specs,
                  check_rep=False),
        donate_argnums=donate, keep_unused=True,
    )

    zshard = tuple(NamedSharding(mesh, PartitionSpec("core")) for _ in range(n_outs))
    zs_shapes = [(n_cores * a.shape[0], *a.shape[1:]) for a in out_avals]
    zs_dtypes = [a.dtype for a in out_avals]

    @jax.jit
    def _zeros():
        return tuple(jnp.zeros(s, d) for s, d in zip(zs_shapes, zs_dtypes))

    _zeros_sharded = jax.jit(_zeros, out_shardings=zshard) if n_outs else _zeros

    in_shard = NamedSharding(mesh, PartitionSpec("core"))
    _pool = ThreadPoolExecutor(n_cores)

    def run(in_maps):
        # on-device zero outputs first (overlaps with uploads)
        zouts = _zeros_sharded()
        # per-device async uploads in device order
        glob_in = []
        for i, name in enumerate(in_names):
            pieces = [jax.device_put(np.asarray(in_maps[c][name]), devices[c])
                      for c in range(n_cores)]
            s0 = pieces[0].shape
            glob_in.append(jax.make_array_from_single_device_arrays(
                (n_cores * s0[0], *s0[1:]), in_shard, pieces))
        out_arrs = sharded(*glob_in, *zouts)
        # threaded per-shard fetch
        fetched = []
        for i in range(n_outs):
            shards = sorted(out_arrs[i].addressable_shards,
                            key=lambda s: s.index[0].start or 0)
            fetched.append(list(_pool.map(lambda s: np.asarray(s.data), shards)))
        return [
            {name: fetched[i][c] for i, name in enumerate(out_names)}
            for c in range(n_cores)
        ]

    return run


def kernel(hidden_states, w_cattn, wq_conv, wk_conv, wv_conv, w_beta,
           o_norm_w, w_o):
    global _NC_CACHE, _WKEY, _RUNNER, _last_in_maps, _LAST_RES

    hidden_states = np.asarray(hidden_states, np.float32)
    w_cattn = np.asarray(w_cattn, np.float32)
    w_beta = np.asarray(w_beta, np.float32)
    w_o = np.asarray(w_o, np.float32)
    o_norm_w = np.asarray(o_norm_w, np.float32)
    convs = [np.asarray(x, np.float32) for x in (wq_conv, wk_conv, wv_conv)]

    wkey = (float(w_cattn[0, 0]), float(w_o[0, 0]), float(w_beta[0, 0]),
            float(convs[0][0, 0]), float(o_norm_w[0]),
            float(w_cattn.sum()), float(w_o.sum()))
    if _NC_CACHE is None or _WKEY != wkey:
        wd = _prep_weights(w_cattn, w_beta, w_o, o_norm_w, convs)
        _NC_CACHE = _build_nc(wd)
        _WKEY = wkey
        _RUNNER = _make_runner(_NC_CACHE)

    # per-core hidden L-quarter: core b*4+hg gets hT[b][:, hg*1024:(hg+1)*1024]
    in_maps = []
    for core in range(8):
        b, hgq = divmod(core, 4)
        hslab = hidden_states[b, hgq * LQ:(hgq + 1) * LQ, :]   # [1024, 2048]
        in_maps.append({"hq": np.ascontiguousarray(hslab.T).astype(BF)})
    _last_in_maps = in_maps

    results = _RUNNER(in_maps)
    _LAST_RES = results
    out = np.empty((B, L, D), np.float32)
    for core in range(8):
        b, hgq = divmod(core, 4)
        out[b, hgq * LQ:(hgq + 1) * LQ, :] = results[core]["yq"]
    return out


if __name__ == "__main__":
    rng = np.random.default_rng(0)
    inputs = {
        "hidden_states": rng.standard_normal((B, L, D), dtype=np.float32),
        "w_cattn": rng.standard_normal((D, 3 * D), dtype=np.float32) * 0.02,
        "wq_conv": rng.standard_normal((D, 4), dtype=np.float32) * 0.3,
        "wk_conv": rng.standard_normal((D, 4), dtype=np.float32) * 0.3,
        "wv_conv": rng.standard_normal((D, 4), dtype=np.float32) * 0.3,
        "w_beta": rng.standard_normal((D, H), dtype=np.float32) * 0.02,
        "o_norm_w": np.ones((DK,), np.float32),
        "w_o": rng.standard_normal((D, D), dtype=np.float32) * 0.02,
    }
    out = kernel(**inputs)
    print("out", out.shape, out.dtype, np.abs(out).max())
